# revision 1
# baseline (speedup 1.0000x reference)
"""Trainium2 Bass kernel for the AttentionModel (word-by-word attention entailment model).

Contract: kernel(**inputs) takes FULL unsharded inputs (as produced by
setup_inputs()) and returns the FULL [512, 2] output. Internally the batch is
sharded over 8 NeuronCores (64 sequences each); the two symmetric branches are
stacked on the partition axis so each core processes 128 "rows"
(row r < 64 -> branch1 seq r, row r >= 64 -> branch2 seq r-64).

Pipeline per core (all on device):
  1. Two LSTMs over 60 steps (slot1 on [x1;x2] with Wx1/Wh1, slot2 on [x2;x1]
     with Wx2/Wh2), producing transposed outputs Y1T/Y2T [h, t, row] and a
     row-major copy Yrh [row, h, l] of slot1's output.
  2. WyY precompute (Y @ W_y, transposed layout).
  3. 60-step attention scan (tmp matmuls on PE, broadcast-add + tanh for M,
     w.M score via M=1 matmul, DRAM-bounce reshape to [row, l], masked
     softmax via exp+accum, alpha-weighted Y sum via mul + tree reduce,
     r update, r_L selection).
  4. Final tanh(r_L@Wp_a + h2@Wxa), branch sum, @U + b_out.

Embedding lookup (x = E[tokens]) and layout shuffles of weights are host-side
preprocessing; all model FLOPs run on device.
"""

import json

import numpy as np


def _split_multi_waits(raw: bytes) -> bytes:
    """Walrus codegen in this toolchain only encodes one sync-wait per
    instruction. Split every instruction carrying N>1 waits into N-1
    standalone EventSemaphore waits (same engine, program order) followed by
    the original instruction keeping a single wait. Sem conditions are
    monotonic, so a sequential wait chain is equivalent to the combined wait.
    """
    j = json.loads(raw)
    uid = [0]
    for fn in j.get("functions", []):
        for blk in fn.get("blocks", []):
            insts = blk.get("instructions", [])
            out = []
            for inst in insts:
                si = inst.get("sync_info")
                waits = (si or {}).get("on_wait") or []
                if len(waits) > 1:
                    eng = inst.get("engine")
                    for w in waits[:-1]:
                        uid[0] += 1
                        out.append({
                            "debug": inst.get("debug", 0),
                            "engine": eng,
                            "ins": [],
                            "outs": [],
                            "name": f"WSPLIT-{uid[0]}",
                            "opcode": "EventSemaphore",
                            "sync_info": {"on_update": [], "on_wait": [w]},
                        })
                    si["on_wait"] = [waits[-1]]
                out.append(inst)
            blk["instructions"] = out
    return json.dumps(j).encode()


def _apply_wait_split(nc):
    import concourse.bass as bass

    patched = _split_multi_waits(bass.Bass.to_json_bytes(nc))
    nc.to_json_bytes = lambda: patched
    return nc


B, L, D, H, V = 512, 60, 300, 256, 50000
NC = 8                 # cores
BC = B // NC           # 64 sequences per core
R = 2 * BC             # 128 rows (2 branches)
H4 = 4 * H             # 1024
DK = 3                 # D split into 3 chunks of 128 (300 + bias row 300 -> padded 384)
LP = 64                # l padded to 64 for the alpha tree
NEG = -10000.0

_cache = {}


def _build_nc():
    import concourse.bass as bass
    import concourse.mybir as mybir
    import concourse.tile as tile
    from concourse.masks import make_identity

    f32 = mybir.dt.float32
    f16 = mybir.dt.float16
    Alu = mybir.AluOpType
    Act = mybir.ActivationFunctionType

    nc = bass.Bass()

    # ---------------- DRAM I/O ----------------
    xT1 = nc.dram_tensor("xT1", [128, L, DK, R], f16, kind="ExternalInput")
    xT2 = nc.dram_tensor("xT2", [128, L, DK, R], f16, kind="ExternalInput")
    Wx1s = nc.dram_tensor("Wx1s", [128, DK, H4], f16, kind="ExternalInput")
    Wx2s = nc.dram_tensor("Wx2s", [128, DK, H4], f16, kind="ExternalInput")
    Wh1s = nc.dram_tensor("Wh1s", [128, 2, H4], f16, kind="ExternalInput")
    Wh2s = nc.dram_tensor("Wh2s", [128, 2, H4], f16, kind="ExternalInput")
    Wys = nc.dram_tensor("Wys", [128, 2, H], f16, kind="ExternalInput")
    Whas = nc.dram_tensor("Whas", [128, 2, H], f16, kind="ExternalInput")
    Wras = nc.dram_tensor("Wras", [128, 2, H], f16, kind="ExternalInput")
    Wtas = nc.dram_tensor("Wtas", [128, 2, H], f16, kind="ExternalInput")
    Wpas = nc.dram_tensor("Wpas", [128, 2, H], f16, kind="ExternalInput")
    Wxas = nc.dram_tensor("Wxas", [128, 2, H], f16, kind="ExternalInput")
    was = nc.dram_tensor("was", [128, 2], f16, kind="ExternalInput")
    Us = nc.dram_tensor("Us", [128, 2, 2], f16, kind="ExternalInput")
    bouts = nc.dram_tensor("bouts", [1, 2], f16, kind="ExternalInput")
    onesb = nc.dram_tensor("onesb", [1, BC], f16, kind="ExternalInput")
    maskadd_d = nc.dram_tensor("maskadd", [R, LP], f16, kind="ExternalInput")
    sel_d = nc.dram_tensor("sel", [R, LP], f32, kind="ExternalInput")
    mf1_d = nc.dram_tensor("mf1", [R, LP], f32, kind="ExternalInput")
    mf2_d = nc.dram_tensor("mf2", [R, LP], f32, kind="ExternalInput")
    out_d = nc.dram_tensor("out", [BC, 2], f32, kind="ExternalOutput")

    with tile.TileContext(nc) as tc:
        with (
            tc.tile_pool(name="persist", bufs=1) as pp,
            tc.tile_pool(name="dram", bufs=1, space="DRAM") as dp,
        ):
            # persistent sbuf tiles
            Y1T = pp.tile([128, 2, L, R], f16)    # slot1 h-state transposed, per t
            Y2T = pp.tile([128, 2, L, R], f16)
            Yrh = pp.tile([128, H, LP], f16)      # slot1 h row-major [row, h, l]
            WyYT = pp.tile([128, 2, L, R], f16)
            wWy = pp.tile([128, 2, H], f16)
            wWha = pp.tile([128, 2, H], f16)
            wWra = pp.tile([128, 2, H], f16)
            wWta = pp.tile([128, 2, H], f16)
            wWpa = pp.tile([128, 2, H], f16)
            wWxa = pp.tile([128, 2, H], f16)
            wwa = pp.tile([128, 2], f16)
            wU = pp.tile([128, 2, 2], f16)
            wbout = pp.tile([1, 2], f16)
            wones = pp.tile([1, BC], f16)
            maskadd = pp.tile([R, LP], f16)
            sel = pp.tile([R, LP], f32)
            mf1 = pp.tile([R, LP], f32)
            mf2 = pp.tile([R, LP], f32)
            ident = pp.tile([128, 128], f32)
            # states
            rr = pp.tile([R, H], f32)             # r (row major)
            rT = pp.tile([128, 2, R], f16)        # r transposed
            rL = pp.tile([R, H], f32)
            uu = pp.tile([R, H], f32)
            s_dram = dp.tile([R * L], f16)        # bridge bounce

            make_identity(nc, ident[:])
            for t_ in (Yrh, rT):
                nc.vector.memset(t_[:], 0.0)
            for t_ in (rr, rL):
                nc.vector.memset(t_[:], 0.0)

            for dst, src in [
                (wWy, Wys), (wWha, Whas), (wWra, Wras), (wWta, Wtas),
                (wWpa, Wpas), (wWxa, Wxas), (wwa, was), (wU, Us),
                (wbout, bouts), (wones, onesb), (maskadd, maskadd_d),
                (sel, sel_d), (mf1, mf1_d), (mf2, mf2_d),
            ]:
                nc.sync.dma_start(dst[:], src[:])

            # ======== Phase 1: the two LSTMs ========
            with (
                tc.tile_pool(name="lstm", bufs=1) as lp,
                tc.tile_pool(name="lstm_x", bufs=3) as lxp,
                tc.tile_pool(name="lstm_ps", bufs=2, space="PSUM") as lps,
                tc.tile_pool(name="tr_ps", bufs=2, space="PSUM") as tps,
            ):
                wWx1 = lp.tile([128, DK, H4], f16)
                wWx2 = lp.tile([128, DK, H4], f16)
                wWh1 = lp.tile([128, 2, H4], f16)
                wWh2 = lp.tile([128, 2, H4], f16)
                nc.sync.dma_start(wWx1[:], Wx1s[:])
                nc.sync.dma_start(wWx2[:], Wx2s[:])
                nc.sync.dma_start(wWh1[:], Wh1s[:])
                nc.sync.dma_start(wWh2[:], Wh2s[:])

                cc = {1: lp.tile([R, H], f32, name="c1"), 2: lp.tile([R, H], f32, name="c2")}
                hh = {1: lp.tile([R, H], f32, name="h1"), 2: lp.tile([R, H], f32, name="h2")}
                for s in (1, 2):
                    nc.vector.memset(cc[s][:], 0.0)
                    nc.vector.memset(hh[s][:], 0.0)

                YT = {1: Y1T, 2: Y2T}
                wWx = {1: wWx1, 2: wWx2}
                wWh = {1: wWh1, 2: wWh2}
                mf = {1: mf1, 2: mf2}
                xTd = {1: xT1, 2: xT2}

                for t in range(L):
                    for s in (1, 2):
                        xt = lxp.tile([128, DK, R], f16, tag="xt")
                        nc.gpsimd.dma_start(xt[:], xTd[s][:, t, :, :])
                        gps = lps.tile([R, H4], f32, tag="gates")
                        for nck in range(2):
                            nsl = slice(nck * 512, (nck + 1) * 512)
                            mms = [(xt[:, dk, :], wWx[s][:, dk, nsl])
                                   for dk in range(DK)]
                            if t > 0:
                                mms += [(YT[s][:, kt, t - 1, :], wWh[s][:, kt, nsl])
                                        for kt in range(2)]
                            for i, (a_, b_) in enumerate(mms):
                                nc.tensor.matmul(
                                    gps[:, nsl], a_, b_,
                                    start=(i == 0), stop=(i == len(mms) - 1))
                        # nonlinearities (gate order i,j,f,o)
                        si = lp.tile([R, H], f32, tag="si")
                        tj = lp.tile([R, H], f32, tag="tj")
                        sf = lp.tile([R, H], f32, tag="sf")
                        so = lp.tile([R, H], f32, tag="so")
                        nc.scalar.activation(si[:], gps[:, 0:256], Act.Sigmoid)
                        nc.scalar.activation(tj[:], gps[:, 256:512], Act.Tanh)
                        nc.scalar.activation(sf[:], gps[:, 512:768], Act.Sigmoid, bias=1.0)
                        nc.scalar.activation(so[:], gps[:, 768:1024], Act.Sigmoid)
                        t1 = lp.tile([R, H], f32, tag="t1")
                        t2 = lp.tile([R, H], f32, tag="t2")
                        cn = lp.tile([R, H], f32, tag="cn")
                        nc.vector.tensor_tensor(t1[:], cc[s][:], sf[:], op=Alu.mult)
                        nc.vector.tensor_tensor(t2[:], si[:], tj[:], op=Alu.mult)
                        nc.vector.tensor_tensor(cn[:], t1[:], t2[:], op=Alu.add)
                        # freeze: c += m*(cn - c)
                        dcv = lp.tile([R, H], f32, tag="dcv")
                        nc.vector.tensor_tensor(dcv[:], cn[:], cc[s][:], op=Alu.subtract)
                        nc.vector.scalar_tensor_tensor(
                            cc[s][:], dcv[:], mf[s][:, t:t + 1], cc[s][:],
                            op0=Alu.mult, op1=Alu.add)
                        tcn = lp.tile([R, H], f32, tag="tcn")
                        nc.scalar.activation(tcn[:], cn[:], Act.Tanh)
                        hn = lp.tile([R, H], f32, tag="hn")
                        nc.vector.tensor_tensor(hn[:], tcn[:], so[:], op=Alu.mult)
                        dhv = lp.tile([R, H], f32, tag="dhv")
                        nc.vector.tensor_tensor(dhv[:], hn[:], hh[s][:], op=Alu.subtract)
                        nc.vector.scalar_tensor_tensor(
                            hh[s][:], dhv[:], mf[s][:, t:t + 1], hh[s][:],
                            op0=Alu.mult, op1=Alu.add)
                        # transpose frozen h into Y{s}T[:, :, t, :]
                        for kt in range(2):
                            tp = tps.tile([128, 128], f32, tag="tp")
                            nc.tensor.transpose(
                                tp[:], hh[s][:, kt * 128:(kt + 1) * 128], ident[:])
                            nc.scalar.copy(YT[s][:, kt, t, :], tp[:])
                        if s == 1:
                            nc.vector.tensor_copy(Yrh[:, :, t], hh[1][:])

            # ======== Phase 2: WyY precompute ========
            NCH = 15  # 7680 / 512
            with (
                tc.tile_pool(name="wyy_ps", bufs=4, space="PSUM") as wps,
            ):
                y1flat = Y1T[:].rearrange("p k l r -> p k (l r)")
                wyflat = WyYT[:].rearrange("p k l r -> p k (l r)")
                for mt in range(2):
                    for c in range(NCH):
                        csl = slice(c * 512, (c + 1) * 512)
                        ps = wps.tile([128, 512], f32, tag="wyy")
                        for kt in range(2):
                            nc.tensor.matmul(
                                ps[:], wWy[:, kt, mt * 128:(mt + 1) * 128],
                                y1flat[:, kt, csl], start=(kt == 0), stop=(kt == 1))
                        if (mt * NCH + c) % 2 == 0:
                            nc.scalar.copy(wyflat[:, mt, csl], ps[:])
                        else:
                            nc.vector.tensor_copy(wyflat[:, mt, csl], ps[:])

            # ======== Phase 3: attention scan ========
            SCH = 16           # score chunks
            SCW = R * L // SCH  # 960
            with (
                tc.tile_pool(name="attn", bufs=1) as ap,
                tc.tile_pool(name="gm", bufs=1) as gmp,
                tc.tile_pool(name="ptree", bufs=1) as ptp,
                tc.tile_pool(name="at_ps", bufs=1, space="PSUM") as aps,
                tc.tile_pool(name="sc_ps", bufs=2, space="PSUM") as sps,
            ):
                GM = gmp.tile([128, 2, L, R], f16)
                e64 = ap.tile([R, LP], f16)
                nc.vector.memset(e64[:], 0.0)
                den = ap.tile([R, 1], f32)
                rden = ap.tile([R, 1], f32)
                s_rl = ap.tile([R, L], f16)
                sm = ap.tile([R, L], f16)
                tmpT = ap.tile([128, 2, R], f16)
                TT = ap.tile([R, H], f32)

                gmflat = GM[:].rearrange("p k l r -> p k (l r)")
                gmrl = GM[:].rearrange("p k l r -> p k r l")  # [128,2,R,L]

                for t in range(L):
                    # --- tmp_T = Wha.T @ h_t_T + Wra.T @ r_T  (transposed) ---
                    for mt in range(2):
                        tps_ = aps.tile([128, R], f32, tag="tmps")
                        msl = slice(mt * 128, (mt + 1) * 128)
                        for kt in range(2):
                            nc.tensor.matmul(
                                tps_[:], wWha[:, kt, msl], Y2T[:, kt, t, :],
                                start=(kt == 0), stop=False)
                        for kt in range(2):
                            nc.tensor.matmul(
                                tps_[:], wWra[:, kt, msl], rT[:, kt, :],
                                start=False, stop=(kt == 1))
                        nc.scalar.copy(tmpT[:, mt, :], tps_[:])
                    # --- rWt (row major) + T = tanh ---
                    rwt = aps.tile([R, H], f32, tag="rwt")
                    for kt in range(2):
                        nc.tensor.matmul(
                            rwt[:], rT[:, kt, :], wWta[:, kt, :],
                            start=(kt == 0), stop=(kt == 1))
                    nc.scalar.activation(TT[:], rwt[:], Act.Tanh)
                    # --- G = WyYT + tmpT (broadcast over l), tanh in place ---
                    for kt in range(2):
                        g_eng = nc.vector if kt == 0 else nc.gpsimd
                        g_eng.tensor_tensor(
                            GM[:, kt, :, :], WyYT[:, kt, :, :],
                            tmpT[:, kt, :].unsqueeze(1).broadcast_to([128, L, R]),
                            op=Alu.add)
                    for kt in range(2):
                        nc.scalar.activation(
                            gmflat[:, kt, :], gmflat[:, kt, :], Act.Tanh)
                    # --- score = w . M  -> psum [1, SCW] chunks -> s_dram ---
                    RB = R // SCH  # rows per score chunk
                    for c in range(SCH):
                        scp = sps.tile([1, SCW], f32, tag="scp")
                        csl = slice(c * SCW, (c + 1) * SCW)
                        for kt in range(2):
                            nc.tensor.matmul(
                                scp[:], wwa[:, kt:kt + 1],
                                gmrl[:, kt, c * RB:(c + 1) * RB, :],
                                start=(kt == 0), stop=(kt == 1))
                        sfl = ap.tile([1, SCW], f16, tag="sfl", bufs=3)
                        if c % 2 == 0:
                            nc.vector.tensor_copy(sfl[:], scp[:])
                        else:
                            nc.scalar.copy(sfl[:], scp[:])
                        nc.gpsimd.dma_start(s_dram[csl], sfl[0:1, :])
                    # --- bounce back as [row, l] ---
                    nc.gpsimd.dma_start(
                        s_rl[:], s_dram[:].rearrange("(r l) -> r l", r=R))
                    # --- masked softmax (unnormalized) ---
                    nc.vector.tensor_tensor(sm[:], s_rl[:], maskadd[:, 0:L], op=Alu.add)
                    nc.scalar.activation(
                        e64[:, 0:L], sm[:], Act.Exp, accum_out=den[:])
                    nc.vector.reciprocal(rden[:], den[:])
                    # --- u = (e . Y) * rden : two h-halves, tree over l ---
                    HQ = 64
                    for hf in range(H // HQ):
                        hsl = slice(hf * HQ, (hf + 1) * HQ)
                        P = ptp.tile([128, HQ, LP], f16, tag="P")
                        # split the heavy multiply+top tree level between DVE
                        # and the otherwise-idle GPSIMD by quarter parity
                        mul_eng = nc.vector if hf % 2 == 0 else nc.gpsimd
                        add_eng = nc.gpsimd if hf % 2 == 0 else nc.vector
                        mul_eng.tensor_tensor(
                            P[:], Yrh[:, hsl, :],
                            e64[:].unsqueeze(1).broadcast_to([R, HQ, LP]),
                            op=Alu.mult)
                        A = ptp.tile([128, HQ, 32], f16, tag="A")
                        add_eng.tensor_tensor(
                            A[:], P[:, :, 0:32], P[:, :, 32:64], op=Alu.add)
                        Bv = ptp.tile([128, HQ, 16], f16, tag="Bv")
                        nc.vector.tensor_tensor(
                            Bv[:], A[:, :, 0:16], A[:, :, 16:32], op=Alu.add)
                        Cv = ptp.tile([128, HQ, 8], f16, tag="Cv")
                        nc.vector.tensor_tensor(
                            Cv[:], Bv[:, :, 0:8], Bv[:, :, 8:16], op=Alu.add)
                        Dv = ptp.tile([128, HQ, 4], f16, tag="Dv")
                        nc.vector.tensor_tensor(
                            Dv[:], Cv[:, :, 0:4], Cv[:, :, 4:8], op=Alu.add)
                        uh = ptp.tile([128, HQ], f32, tag="uh")
                        nc.vector.tensor_reduce(
                            uh[:], Dv[:], axis=mybir.AxisListType.X, op=Alu.add)
                        nc.vector.tensor_scalar_mul(uu[:, hsl], uh[:], rden[:])
                    # --- r = u + T ; r_L += sel_t * r ; transpose r ---
                    nc.vector.tensor_tensor(rr[:], uu[:], TT[:], op=Alu.add)
                    nc.vector.scalar_tensor_tensor(
                        rL[:], rr[:], sel[:, t:t + 1], rL[:],
                        op0=Alu.mult, op1=Alu.add)
                    for kt in range(2):
                        tp = aps.tile([128, 128], f32, tag="rtp")
                        nc.tensor.transpose(
                            tp[:], rr[:, kt * 128:(kt + 1) * 128], ident[:])
                        nc.scalar.copy(rT[:, kt, :], tp[:])

                # ======== Phase 4: final head ========
                rLT = ap.tile([128, 2, R], f16)
                for kt in range(2):
                    tp = aps.tile([128, 128], f32, tag="rtp")
                    nc.tensor.transpose(
                        tp[:], rL[:, kt * 128:(kt + 1) * 128], ident[:])
                    nc.scalar.copy(rLT[:, kt, :], tp[:])
                fT = ap.tile([128, 2, R], f16)
                for mt in range(2):
                    msl = slice(mt * 128, (mt + 1) * 128)
                    fps = aps.tile([128, R], f32, tag="fps")
                    for kt in range(2):
                        nc.tensor.matmul(
                            fps[:], wWpa[:, kt, msl], rLT[:, kt, :],
                            start=(kt == 0), stop=False)
                    for kt in range(2):
                        nc.tensor.matmul(
                            fps[:], wWxa[:, kt, msl], Y2T[:, kt, L - 1, :],
                            start=False, stop=(kt == 1))
                    nc.scalar.activation(fT[:, mt, :], fps[:], Act.Tanh)
                lhT = ap.tile([128, 2, BC], f16)
                nc.vector.tensor_tensor(
                    lhT[:], fT[:, :, 0:BC], fT[:, :, BC:R], op=Alu.add)
                ops_ = aps.tile([BC, 2], f32, tag="ops")
                for kt in range(2):
                    nc.tensor.matmul(
                        ops_[:], lhT[:, kt, :], wU[:, kt, :],
                        start=(kt == 0), stop=False)
                nc.tensor.matmul(ops_[:], wones[:], wbout[:], start=False, stop=True)
                osb = ap.tile([BC, 2], f32)
                nc.vector.tensor_copy(osb[:], ops_[:])
                nc.sync.dma_start(out_d[:], osb[:])

    return _apply_wait_split(nc)


def _pack_w2(W):
    # [256, N] -> [128, 2, N]
    return np.stack([W[0:128], W[128:256]], axis=1)


def _prep_inputs(E, Wx1, Wh1, b1, Wx2, Wh2, b2, W_y, Wh_a, Wr_a, w_a, Wt_a,
                 Wp_a, Wxa, U, b_out, input1, input2, seqlen1, seqlen2):
    """Build the per-core input maps (host-side sharding + layout packing)."""
    E = np.asarray(E, np.float32)
    f16 = np.float16
    common = {}

    def packx(Wx, b):
        Wa = np.zeros((128, DK, H4), np.float32)
        Wa[:, 0, :] = Wx[0:128]
        Wa[:, 1, :] = Wx[128:256]
        Wa[0:44, 2, :] = Wx[256:300]
        Wa[44, 2, :] = b  # bias row, matched by the ones-row in xT
        return Wa.astype(f16)

    common["Wx1s"] = packx(np.asarray(Wx1, np.float32), np.asarray(b1, np.float32))
    common["Wx2s"] = packx(np.asarray(Wx2, np.float32), np.asarray(b2, np.float32))
    common["Wh1s"] = _pack_w2(np.asarray(Wh1, np.float32)).astype(f16)
    common["Wh2s"] = _pack_w2(np.asarray(Wh2, np.float32)).astype(f16)
    for nm, W in [("Wys", W_y), ("Whas", Wh_a), ("Wras", Wr_a), ("Wtas", Wt_a),
                  ("Wpas", Wp_a), ("Wxas", Wxa)]:
        common[nm] = _pack_w2(np.asarray(W, np.float32)).astype(f16)
    wa = np.asarray(w_a, np.float32)
    common["was"] = np.stack([wa[0:128], wa[128:256]], 1).astype(f16)
    common["Us"] = _pack_w2(np.asarray(U, np.float32)).astype(f16)
    common["bouts"] = np.asarray(b_out, np.float32).reshape(1, 2).astype(f16)
    common["onesb"] = np.ones((1, BC), f16)

    input1 = np.asarray(input1)
    input2 = np.asarray(input2)
    seqlen1 = np.asarray(seqlen1)
    seqlen2 = np.asarray(seqlen2)

    in_maps = []
    for c in range(NC):
        sl = slice(c * BC, (c + 1) * BC)
        t1, t2 = input1[sl], input2[sl]
        s1, s2 = seqlen1[sl], seqlen2[sl]
        stack1 = np.concatenate([t1, t2], 0)   # [128, 60] tokens, slot1
        stack2 = np.concatenate([t2, t1], 0)
        lf = np.concatenate([s1, s2], 0)       # len of first-arg seq per row
        ls = np.concatenate([s2, s1], 0)       # len of second-arg seq per row

        def pack_xT(stack):
            x = E[stack]                        # [128, 60, 300]
            xT = np.zeros((128, L, DK, R), np.float32)
            xt = np.transpose(x, (2, 1, 0))     # [300, 60, 128]
            xT[:, :, 0, :] = xt[0:128]
            xT[:, :, 1, :] = xt[128:256]
            xT[0:44, :, 2, :] = xt[256:300]
            xT[44, :, 2, :] = 1.0               # bias ones-row
            return xT.astype(f16)

        m = {}
        m["xT1"] = pack_xT(stack1)
        m["xT2"] = pack_xT(stack2)
        ar = np.arange(L)[None, :]
        m["maskadd"] = np.where(ar < lf[:, None], 0.0, NEG).astype(np.float32)
        m["maskadd"] = np.concatenate(
            [m["maskadd"], np.full((R, LP - L), NEG, np.float32)], 1).astype(f16)
        selm = (ar == (ls[:, None] - 1)).astype(np.float32)
        m["sel"] = np.concatenate([selm, np.zeros((R, LP - L), np.float32)], 1)
        mk1 = (ar < lf[:, None]).astype(np.float32)
        mk2 = (ar < ls[:, None]).astype(np.float32)
        m["mf1"] = np.concatenate([mk1, np.zeros((R, LP - L), np.float32)], 1)
        m["mf2"] = np.concatenate([mk2, np.zeros((R, LP - L), np.float32)], 1)
        m.update(common)
        in_maps.append(m)
    return in_maps


_last_exec_ns = None


def kernel(__trace=False, **inputs):
    global _last_exec_ns
    from concourse.bass_utils import run_bass_kernel_spmd

    if "nc" not in _cache:
        _cache["nc"] = _build_nc()
    nc = _cache["nc"]
    in_maps = _prep_inputs(**inputs)
    res = run_bass_kernel_spmd(nc, in_maps, core_ids=list(range(NC)),
                               trace=__trace)
    if getattr(res, "exec_time_ns", None):
        _last_exec_ns = res.exec_time_ns
    out = np.concatenate([r["out"] for r in res.results], axis=0)
    return out.astype(np.float32)



# revision 5
# speedup vs baseline: 1.8267x; 1.8267x over previous
"""Trainium2 Bass kernel for the AttentionModel (word-by-word attention entailment model).

Contract: kernel(**inputs) takes FULL unsharded inputs (as produced by
setup_inputs()) and returns the FULL [512, 2] output. Internally the batch is
sharded over 8 NeuronCores (64 sequences each); the two symmetric branches are
stacked on the partition axis so each core processes 128 "rows"
(row r < 64 -> branch1 seq r, row r >= 64 -> branch2 seq r-64).

The end-to-end call is dominated by host->device transfer over the axon
tunnel (~40-50 MB/s), so the input payload is minimized:
  * Only slot-1 embeddings are sent, row-major f16 [128, L, 300] per core
    (4.6 MB). Slot 2's stack ([x2;x1]) is the same data with the row axis
    rotated by 64, derived on device. The dims-major layout the PE needs is
    produced by on-device PE transposes (3 per step).
  * All weights are packed into one flat f16 buffer; each core uploads 1/8
    and the full buffer is reassembled on device with an AllGather.
  * Freeze masks / softmax mask / last-step selector are computed on device
    with iota + per-partition compares from a tiny [128, 2] seqlen input.

Pipeline per core (all on device):
  1. Two LSTMs over 60 steps (slot1 on [x1;x2] with Wx1/Wh1, slot2 on [x2;x1]
     with Wx2/Wh2), producing transposed outputs Y1T/Y2T [h, t, row] and a
     row-major copy Yrh [row, h, l] of slot1's output.
  2. WyY precompute (Y @ W_y, transposed layout).
  3. 60-step attention scan (tmp matmuls on PE, broadcast-add + tanh for M,
     w.M score via M=1 matmul, DRAM-bounce reshape to [row, l], masked
     softmax via exp+accum, alpha-weighted Y sum via mul + tree reduce,
     r update, r_L selection).
  4. Final tanh(r_L@Wp_a + h2@Wxa), branch sum, @U + b_out.
"""

import json

import numpy as np


def _split_multi_waits(raw: bytes) -> bytes:
    """Walrus codegen in this toolchain only encodes one sync-wait per
    instruction. Split every instruction carrying N>1 waits into N-1
    standalone EventSemaphore waits (same engine, program order) followed by
    the original instruction keeping a single wait. Sem conditions are
    monotonic, so a sequential wait chain is equivalent to the combined wait.
    """
    j = json.loads(raw)
    uid = [0]
    for fn in j.get("functions", []):
        for blk in fn.get("blocks", []):
            insts = blk.get("instructions", [])
            out = []
            for inst in insts:
                si = inst.get("sync_info")
                waits = (si or {}).get("on_wait") or []
                if len(waits) > 1:
                    eng = inst.get("engine")
                    for w in waits[:-1]:
                        uid[0] += 1
                        out.append({
                            "debug": inst.get("debug", 0),
                            "engine": eng,
                            "ins": [],
                            "outs": [],
                            "name": f"WSPLIT-{uid[0]}",
                            "opcode": "EventSemaphore",
                            "sync_info": {"on_update": [], "on_wait": [w]},
                        })
                    si["on_wait"] = [waits[-1]]
                out.append(inst)
            blk["instructions"] = out
    return json.dumps(j).encode()


def _apply_wait_split(nc):
    import concourse.bass as bass

    patched = _split_multi_waits(bass.Bass.to_json_bytes(nc))
    nc.to_json_bytes = lambda: patched
    return nc


B, L, D, H, V = 512, 60, 300, 256, 50000
NC = 8                 # cores
BC = B // NC           # 64 sequences per core
R = 2 * BC             # 128 rows (2 branches)
H4 = 4 * H             # 1024
DB = 45                # third d-chunk: rows 256..299 + bias ones-row at 44
LP = 64                # l padded to 64 for the alpha tree
NEG = -10000.0

# flat weight buffer layout (f16 elems); uploaded sharded + AllGathered
_WSPECS = [
    ("Wx1A", (128, 2, H4)), ("Wx2A", (128, 2, H4)),
    ("Wx1B", (DB, H4)), ("Wx2B", (DB, H4)),
    ("Wh1", (128, 2, H4)), ("Wh2", (128, 2, H4)),
    ("Wy", (128, 2, H)), ("Wha", (128, 2, H)), ("Wra", (128, 2, H)),
    ("Wta", (128, 2, H)), ("Wpa", (128, 2, H)), ("Wxa", (128, 2, H)),
    ("wa", (128, 2)), ("U", (128, 2, 2)), ("bout", (1, 2)),
]
_WOFF = {}
_off = 0
for _nm, _shp in _WSPECS:
    _WOFF[_nm] = _off
    _n = 1
    for _d in _shp:
        _n *= _d
    _off += _n
SH = 192000            # per-core weight shard elems
SW = SH * NC           # padded flat weight buffer elems
assert _off <= SW

_cache = {}


def _build_nc():
    import concourse.bass as bass
    import concourse.mybir as mybir
    import concourse.tile as tile
    from concourse.masks import make_identity

    f32 = mybir.dt.float32
    f16 = mybir.dt.float16
    Alu = mybir.AluOpType
    Act = mybir.ActivationFunctionType

    nc = bass.Bass()

    # ---------------- DRAM I/O ----------------
    x1_d = nc.dram_tensor("x1", [R, L, D], f16, kind="ExternalInput")
    wsh_d = nc.dram_tensor("wsh", [SH], f16, kind="ExternalInput")
    sl_d = nc.dram_tensor("sl", [R, 2], f32, kind="ExternalInput")
    out_d = nc.dram_tensor("out", [BC, 2], f32, kind="ExternalOutput")

    with tile.TileContext(nc) as tc:
        with (
            tc.tile_pool(name="persist", bufs=1) as pp,
            tc.tile_pool(name="dram", bufs=1, space="DRAM") as dp,
        ):
            # ---- weight shard upload + AllGather into the full flat buffer
            wshard = dp.tile([SH], f16)
            wfull = dp.tile([SW], f16)
            nc.gpsimd.dma_start(wshard[:], wsh_d[:])
            nc.gpsimd.collective_compute(
                "AllGather", mybir.AluOpType.bypass,
                replica_groups=[list(range(NC))],
                ins=[wshard[:].opt()], outs=[wfull[:].opt()])

            def wslice(name):
                off = _WOFF[name]
                shp = dict(_WSPECS)[name]
                n = 1
                for d_ in shp:
                    n *= d_
                ap = wfull[off:off + n]
                if len(shp) == 2:
                    return ap.rearrange("(p n) -> p n", p=shp[0])
                return ap.rearrange("(p k n) -> p k n", p=shp[0], k=shp[1])

            # persistent sbuf tiles
            Y1T = pp.tile([128, 2, L, R], f16)    # slot1 h-state transposed, per t
            Y2T = pp.tile([128, 2, L, R], f16)
            Yrh = pp.tile([128, H, LP], f16)      # slot1 h row-major [row, h, l]
            WyYT = pp.tile([128, 2, L, R], f16)
            wWy = pp.tile([128, 2, H], f16)
            wWha = pp.tile([128, 2, H], f16)
            wWra = pp.tile([128, 2, H], f16)
            wWta = pp.tile([128, 2, H], f16)
            wWpa = pp.tile([128, 2, H], f16)
            wWxa = pp.tile([128, 2, H], f16)
            wwa = pp.tile([128, 2], f16)
            wU = pp.tile([128, 2, 2], f16)
            wbout = pp.tile([1, 2], f16)
            wones = pp.tile([1, BC], f16)
            sl_sb = pp.tile([R, 2], f32)
            lio = pp.tile([R, LP], f32)
            maskadd = pp.tile([R, LP], f16)
            sel = pp.tile([R, LP], f32)
            mf1 = pp.tile([R, LP], f32)
            mf2 = pp.tile([R, LP], f32)
            ident = pp.tile([128, 128], f32)
            ident16 = pp.tile([128, 128], f16)
            # states
            rr = pp.tile([R, H], f32)             # r (row major)
            rT = pp.tile([128, 2, R], f16)        # r transposed
            rL = pp.tile([R, H], f32)
            uu = pp.tile([R, H], f32)
            s_dram = dp.tile([R * L], f16)        # bridge bounce

            make_identity(nc, ident[:])
            make_identity(nc, ident16[:])
            for t_ in (Yrh, rT):
                nc.vector.memset(t_[:], 0.0)
            for t_ in (rr, rL):
                nc.vector.memset(t_[:], 0.0)
            nc.vector.memset(wones[:], 1.0)

            for dst, nm in [
                (wWy, "Wy"), (wWha, "Wha"), (wWra, "Wra"), (wWta, "Wta"),
                (wWpa, "Wpa"), (wWxa, "Wxa"), (wwa, "wa"), (wU, "U"),
                (wbout, "bout"),
            ]:
                nc.sync.dma_start(dst[:], wslice(nm))

            # ---- masks from seqlens: lf = sl[:,0], ls-1 = sl[:,1]
            nc.sync.dma_start(sl_sb[:], sl_d[:])
            nc.gpsimd.iota(lio[:], pattern=[[1, LP]], base=0,
                           channel_multiplier=0,
                           allow_small_or_imprecise_dtypes=True)
            nc.vector.tensor_scalar(
                mf1[:], lio[:], sl_sb[:, 0:1], None, op0=Alu.is_lt)
            nc.vector.tensor_scalar(
                mf2[:], lio[:], sl_sb[:, 1:2], None, op0=Alu.is_le)
            nc.vector.tensor_scalar(
                maskadd[:], lio[:], sl_sb[:, 0:1], NEG,
                op0=Alu.is_ge, op1=Alu.mult)
            nc.vector.tensor_scalar(
                sel[:], lio[:], sl_sb[:, 1:2], None, op0=Alu.is_equal)

            # ======== Phase 1: the two LSTMs ========
            with (
                tc.tile_pool(name="lstm", bufs=1) as lp,
                tc.tile_pool(name="lstm_x", bufs=3) as lxp,
                tc.tile_pool(name="lstm_xt", bufs=2) as lxt,
                tc.tile_pool(name="lstm_ps", bufs=2, space="PSUM") as lps,
                tc.tile_pool(name="tr_ps", bufs=2, space="PSUM") as tps,
                tc.tile_pool(name="xtr_ps", bufs=2, space="PSUM") as xps,
            ):
                wWx1A = lp.tile([128, 2, H4], f16, name="wx1a")
                wWx2A = lp.tile([128, 2, H4], f16, name="wx2a")
                wWx1B = lp.tile([DB, H4], f16, name="wx1b")
                wWx2B = lp.tile([DB, H4], f16, name="wx2b")
                wWh1 = lp.tile([128, 2, H4], f16, name="wh1")
                wWh2 = lp.tile([128, 2, H4], f16, name="wh2")
                for dst, nm in [(wWx1A, "Wx1A"), (wWx2A, "Wx2A"),
                                (wWx1B, "Wx1B"), (wWx2B, "Wx2B"),
                                (wWh1, "Wh1"), (wWh2, "Wh2")]:
                    nc.sync.dma_start(dst[:], wslice(nm))

                cc = {1: lp.tile([R, H], f32, name="c1"), 2: lp.tile([R, H], f32, name="c2")}
                hh = {1: lp.tile([R, H], f32, name="h1"), 2: lp.tile([R, H], f32, name="h2")}
                for s in (1, 2):
                    nc.vector.memset(cc[s][:], 0.0)
                    nc.vector.memset(hh[s][:], 0.0)

                YT = {1: Y1T, 2: Y2T}
                wWxA = {1: wWx1A, 2: wWx2A}
                wWxB = {1: wWx1B, 2: wWx2B}
                wWh = {1: wWh1, 2: wWh2}
                mf = {1: mf1, 2: mf2}

                # pre-set the bias ones-row (44) in both xb1 pool buffers;
                # per-step writes only touch rows 0:44 so it persists, and
                # xb2's rotated copy carries it over
                for _ in range(2):
                    b_ = lxt.tile([DB, R], f16, tag="xb1")
                    nc.vector.memset(b_[:], 1.0)

                for t in range(L):
                    # slot-1 x_t: DMA row-major then PE-transpose to dims-major
                    xr = lxp.tile([R, D], f16, tag="xr")
                    nc.gpsimd.dma_start(xr[:], x1_d[:, t, :])
                    xt1 = lxt.tile([128, 2, R], f16, tag="xt1")
                    xb1 = lxt.tile([DB, R], f16, tag="xb1")
                    for dk in range(2):
                        tp = xps.tile([128, 128], f16, tag="xtp")
                        nc.tensor.transpose(
                            tp[:], xr[:, dk * 128:(dk + 1) * 128], ident16[:])
                        nc.scalar.copy(xt1[:, dk, :], tp[:])
                    tp = xps.tile([128, 128], f16, tag="xtp")
                    nc.tensor.transpose(tp[0:44, :], xr[:, 256:300], ident16[:])
                    nc.scalar.copy(xb1[0:44, :], tp[0:44, :])
                    # slot-2 x_t = slot-1 rotated by 64 on the row axis
                    xt2 = lxt.tile([128, 2, R], f16, tag="xt2")
                    xb2 = lxt.tile([DB, R], f16, tag="xb2")
                    nc.gpsimd.tensor_copy(xt2[:, :, 0:BC], xt1[:, :, BC:R])
                    nc.gpsimd.tensor_copy(xt2[:, :, BC:R], xt1[:, :, 0:BC])
                    nc.gpsimd.tensor_copy(xb2[:, 0:BC], xb1[:, BC:R])
                    nc.gpsimd.tensor_copy(xb2[:, BC:R], xb1[:, 0:BC])
                    xts = {1: xt1, 2: xt2}
                    xbs = {1: xb1, 2: xb2}
                    for s in (1, 2):
                        gps = lps.tile([R, H4], f32, tag="gates")
                        for nck in range(2):
                            nsl = slice(nck * 512, (nck + 1) * 512)
                            mms = [(xts[s][:, 0, :], wWxA[s][:, 0, nsl]),
                                   (xts[s][:, 1, :], wWxA[s][:, 1, nsl]),
                                   (xbs[s][:, :], wWxB[s][:, nsl])]
                            if t > 0:
                                mms += [(YT[s][:, kt, t - 1, :], wWh[s][:, kt, nsl])
                                        for kt in range(2)]
                            for i, (a_, b_) in enumerate(mms):
                                nc.tensor.matmul(
                                    gps[:, nsl], a_, b_,
                                    start=(i == 0), stop=(i == len(mms) - 1))
                        # nonlinearities (gate order i,j,f,o)
                        si = lp.tile([R, H], f32, tag="si")
                        tj = lp.tile([R, H], f32, tag="tj")
                        sf = lp.tile([R, H], f32, tag="sf")
                        so = lp.tile([R, H], f32, tag="so")
                        nc.scalar.activation(si[:], gps[:, 0:256], Act.Sigmoid)
                        nc.scalar.activation(tj[:], gps[:, 256:512], Act.Tanh)
                        nc.scalar.activation(sf[:], gps[:, 512:768], Act.Sigmoid, bias=1.0)
                        nc.scalar.activation(so[:], gps[:, 768:1024], Act.Sigmoid)
                        t1 = lp.tile([R, H], f32, tag="t1")
                        t2 = lp.tile([R, H], f32, tag="t2")
                        cn = lp.tile([R, H], f32, tag="cn")
                        nc.vector.tensor_tensor(t1[:], cc[s][:], sf[:], op=Alu.mult)
                        nc.vector.tensor_tensor(t2[:], si[:], tj[:], op=Alu.mult)
                        nc.vector.tensor_tensor(cn[:], t1[:], t2[:], op=Alu.add)
                        # freeze: c += m*(cn - c)
                        dcv = lp.tile([R, H], f32, tag="dcv")
                        nc.vector.tensor_tensor(dcv[:], cn[:], cc[s][:], op=Alu.subtract)
                        nc.vector.scalar_tensor_tensor(
                            cc[s][:], dcv[:], mf[s][:, t:t + 1], cc[s][:],
                            op0=Alu.mult, op1=Alu.add)
                        tcn = lp.tile([R, H], f32, tag="tcn")
                        nc.scalar.activation(tcn[:], cn[:], Act.Tanh)
                        hn = lp.tile([R, H], f32, tag="hn")
                        nc.vector.tensor_tensor(hn[:], tcn[:], so[:], op=Alu.mult)
                        dhv = lp.tile([R, H], f32, tag="dhv")
                        nc.vector.tensor_tensor(dhv[:], hn[:], hh[s][:], op=Alu.subtract)
                        nc.vector.scalar_tensor_tensor(
                            hh[s][:], dhv[:], mf[s][:, t:t + 1], hh[s][:],
                            op0=Alu.mult, op1=Alu.add)
                        # transpose frozen h into Y{s}T[:, :, t, :]
                        for kt in range(2):
                            tp = tps.tile([128, 128], f32, tag="tp")
                            nc.tensor.transpose(
                                tp[:], hh[s][:, kt * 128:(kt + 1) * 128], ident[:])
                            nc.scalar.copy(YT[s][:, kt, t, :], tp[:])
                        if s == 1:
                            nc.vector.tensor_copy(Yrh[:, :, t], hh[1][:])

            # ======== Phase 2: WyY precompute ========
            NCH = 15  # 7680 / 512
            with (
                tc.tile_pool(name="wyy_ps", bufs=4, space="PSUM") as wps,
            ):
                y1flat = Y1T[:].rearrange("p k l r -> p k (l r)")
                wyflat = WyYT[:].rearrange("p k l r -> p k (l r)")
                for mt in range(2):
                    for c in range(NCH):
                        csl = slice(c * 512, (c + 1) * 512)
                        ps = wps.tile([128, 512], f32, tag="wyy")
                        for kt in range(2):
                            nc.tensor.matmul(
                                ps[:], wWy[:, kt, mt * 128:(mt + 1) * 128],
                                y1flat[:, kt, csl], start=(kt == 0), stop=(kt == 1))
                        if (mt * NCH + c) % 2 == 0:
                            nc.scalar.copy(wyflat[:, mt, csl], ps[:])
                        else:
                            nc.vector.tensor_copy(wyflat[:, mt, csl], ps[:])

            # ======== Phase 3: attention scan ========
            SCH = 16           # score chunks
            SCW = R * L // SCH  # 480
            with (
                tc.tile_pool(name="attn", bufs=1) as ap,
                tc.tile_pool(name="gm", bufs=1) as gmp,
                tc.tile_pool(name="ptree", bufs=1) as ptp,
                tc.tile_pool(name="at_ps", bufs=1, space="PSUM") as aps,
                tc.tile_pool(name="sc_ps", bufs=2, space="PSUM") as sps,
            ):
                GM = gmp.tile([128, 2, L, R], f16)
                e64 = ap.tile([R, LP], f16)
                nc.vector.memset(e64[:], 0.0)
                den = ap.tile([R, 1], f32)
                rden = ap.tile([R, 1], f32)
                s_rl = ap.tile([R, L], f16)
                sm = ap.tile([R, L], f16)
                tmpT = ap.tile([128, 2, R], f16)
                TT = ap.tile([R, H], f32)

                gmflat = GM[:].rearrange("p k l r -> p k (l r)")
                gmrl = GM[:].rearrange("p k l r -> p k r l")  # [128,2,R,L]

                for t in range(L):
                    # --- tmp_T = Wha.T @ h_t_T + Wra.T @ r_T  (transposed) ---
                    for mt in range(2):
                        tps_ = aps.tile([128, R], f32, tag="tmps")
                        msl = slice(mt * 128, (mt + 1) * 128)
                        for kt in range(2):
                            nc.tensor.matmul(
                                tps_[:], wWha[:, kt, msl], Y2T[:, kt, t, :],
                                start=(kt == 0), stop=False)
                        for kt in range(2):
                            nc.tensor.matmul(
                                tps_[:], wWra[:, kt, msl], rT[:, kt, :],
                                start=False, stop=(kt == 1))
                        nc.scalar.copy(tmpT[:, mt, :], tps_[:])
                    # --- rWt (row major) + T = tanh ---
                    rwt = aps.tile([R, H], f32, tag="rwt")
                    for kt in range(2):
                        nc.tensor.matmul(
                            rwt[:], rT[:, kt, :], wWta[:, kt, :],
                            start=(kt == 0), stop=(kt == 1))
                    nc.scalar.activation(TT[:], rwt[:], Act.Tanh)
                    # --- G = WyYT + tmpT (broadcast over l), tanh in place ---
                    for kt in range(2):
                        g_eng = nc.vector if kt == 0 else nc.gpsimd
                        g_eng.tensor_tensor(
                            GM[:, kt, :, :], WyYT[:, kt, :, :],
                            tmpT[:, kt, :].unsqueeze(1).broadcast_to([128, L, R]),
                            op=Alu.add)
                    for kt in range(2):
                        nc.scalar.activation(
                            gmflat[:, kt, :], gmflat[:, kt, :], Act.Tanh)
                    # --- score = w . M  -> psum [1, SCW] chunks -> s_dram ---
                    RB = R // SCH  # rows per score chunk
                    for c in range(SCH):
                        scp = sps.tile([1, SCW], f32, tag="scp")
                        csl = slice(c * SCW, (c + 1) * SCW)
                        for kt in range(2):
                            nc.tensor.matmul(
                                scp[:], wwa[:, kt:kt + 1],
                                gmrl[:, kt, c * RB:(c + 1) * RB, :],
                                start=(kt == 0), stop=(kt == 1))
                        sfl = ap.tile([1, SCW], f16, tag="sfl", bufs=3)
                        if c % 2 == 0:
                            nc.vector.tensor_copy(sfl[:], scp[:])
                        else:
                            nc.scalar.copy(sfl[:], scp[:])
                        nc.gpsimd.dma_start(s_dram[csl], sfl[0:1, :])
                    # --- bounce back as [row, l] ---
                    nc.gpsimd.dma_start(
                        s_rl[:], s_dram[:].rearrange("(r l) -> r l", r=R))
                    # --- masked softmax (unnormalized) ---
                    nc.vector.tensor_tensor(sm[:], s_rl[:], maskadd[:, 0:L], op=Alu.add)
                    nc.scalar.activation(
                        e64[:, 0:L], sm[:], Act.Exp, accum_out=den[:])
                    nc.vector.reciprocal(rden[:], den[:])
                    # --- u = (e . Y) * rden : two h-halves, tree over l ---
                    HQ = 64
                    for hf in range(H // HQ):
                        hsl = slice(hf * HQ, (hf + 1) * HQ)
                        P = ptp.tile([128, HQ, LP], f16, tag="P")
                        # split the heavy multiply+top tree level between DVE
                        # and the otherwise-idle GPSIMD by quarter parity
                        mul_eng = nc.vector if hf % 2 == 0 else nc.gpsimd
                        add_eng = nc.gpsimd if hf % 2 == 0 else nc.vector
                        mul_eng.tensor_tensor(
                            P[:], Yrh[:, hsl, :],
                            e64[:].unsqueeze(1).broadcast_to([R, HQ, LP]),
                            op=Alu.mult)
                        A = ptp.tile([128, HQ, 32], f16, tag="A")
                        add_eng.tensor_tensor(
                            A[:], P[:, :, 0:32], P[:, :, 32:64], op=Alu.add)
                        Bv = ptp.tile([128, HQ, 16], f16, tag="Bv")
                        nc.vector.tensor_tensor(
                            Bv[:], A[:, :, 0:16], A[:, :, 16:32], op=Alu.add)
                        Cv = ptp.tile([128, HQ, 8], f16, tag="Cv")
                        nc.vector.tensor_tensor(
                            Cv[:], Bv[:, :, 0:8], Bv[:, :, 8:16], op=Alu.add)
                        Dv = ptp.tile([128, HQ, 4], f16, tag="Dv")
                        nc.vector.tensor_tensor(
                            Dv[:], Cv[:, :, 0:4], Cv[:, :, 4:8], op=Alu.add)
                        uh = ptp.tile([128, HQ], f32, tag="uh")
                        nc.vector.tensor_reduce(
                            uh[:], Dv[:], axis=mybir.AxisListType.X, op=Alu.add)
                        nc.vector.tensor_scalar_mul(uu[:, hsl], uh[:], rden[:])
                    # --- r = u + T ; r_L += sel_t * r ; transpose r ---
                    nc.vector.tensor_tensor(rr[:], uu[:], TT[:], op=Alu.add)
                    nc.vector.scalar_tensor_tensor(
                        rL[:], rr[:], sel[:, t:t + 1], rL[:],
                        op0=Alu.mult, op1=Alu.add)
                    for kt in range(2):
                        tp = aps.tile([128, 128], f32, tag="rtp")
                        nc.tensor.transpose(
                            tp[:], rr[:, kt * 128:(kt + 1) * 128], ident[:])
                        nc.scalar.copy(rT[:, kt, :], tp[:])

                # ======== Phase 4: final head ========
                rLT = ap.tile([128, 2, R], f16)
                for kt in range(2):
                    tp = aps.tile([128, 128], f32, tag="rtp")
                    nc.tensor.transpose(
                        tp[:], rL[:, kt * 128:(kt + 1) * 128], ident[:])
                    nc.scalar.copy(rLT[:, kt, :], tp[:])
                fT = ap.tile([128, 2, R], f16)
                for mt in range(2):
                    msl = slice(mt * 128, (mt + 1) * 128)
                    fps = aps.tile([128, R], f32, tag="fps")
                    for kt in range(2):
                        nc.tensor.matmul(
                            fps[:], wWpa[:, kt, msl], rLT[:, kt, :],
                            start=(kt == 0), stop=False)
                    for kt in range(2):
                        nc.tensor.matmul(
                            fps[:], wWxa[:, kt, msl], Y2T[:, kt, L - 1, :],
                            start=False, stop=(kt == 1))
                    nc.scalar.activation(fT[:, mt, :], fps[:], Act.Tanh)
                lhT = ap.tile([128, 2, BC], f16)
                nc.vector.tensor_tensor(
                    lhT[:], fT[:, :, 0:BC], fT[:, :, BC:R], op=Alu.add)
                ops_ = aps.tile([BC, 2], f32, tag="ops")
                for kt in range(2):
                    nc.tensor.matmul(
                        ops_[:], lhT[:, kt, :], wU[:, kt, :],
                        start=(kt == 0), stop=False)
                nc.tensor.matmul(ops_[:], wones[:], wbout[:], start=False, stop=True)
                osb = ap.tile([BC, 2], f32)
                nc.vector.tensor_copy(osb[:], ops_[:])
                nc.sync.dma_start(out_d[:], osb[:])

    return _apply_wait_split(nc)


def _prep_inputs(E, Wx1, Wh1, b1, Wx2, Wh2, b2, W_y, Wh_a, Wr_a, w_a, Wt_a,
                 Wp_a, Wxa, U, b_out, input1, input2, seqlen1, seqlen2):
    """Build the per-core input maps (host-side sharding + packing)."""
    f16 = np.float16
    E16 = np.asarray(E, np.float32).astype(f16)

    def pack_w2(W):
        W = np.asarray(W, np.float32)
        return np.stack([W[0:128], W[128:256]], axis=1).astype(f16)

    def packB(W, b):
        W = np.asarray(W, np.float32)
        out = np.zeros((DB, H4), np.float32)
        out[0:44] = W[256:300]
        out[44] = np.asarray(b, np.float32)  # bias row, matched by ones-row
        return out.astype(f16)

    wa = np.asarray(w_a, np.float32)
    parts = [
        pack_w2(Wx1).ravel(), pack_w2(Wx2).ravel(),
        packB(Wx1, b1).ravel(), packB(Wx2, b2).ravel(),
        pack_w2(Wh1).ravel(), pack_w2(Wh2).ravel(),
        pack_w2(W_y).ravel(), pack_w2(Wh_a).ravel(), pack_w2(Wr_a).ravel(),
        pack_w2(Wt_a).ravel(), pack_w2(Wp_a).ravel(), pack_w2(Wxa).ravel(),
        np.stack([wa[0:128], wa[128:256]], 1).astype(f16).ravel(),
        pack_w2(U).ravel(),
        np.asarray(b_out, np.float32).reshape(1, 2).astype(f16).ravel(),
    ]
    wflat = np.concatenate(parts)
    assert wflat.size == _off
    wflat = np.concatenate([wflat, np.zeros(SW - wflat.size, f16)])

    input1 = np.asarray(input1)
    input2 = np.asarray(input2)
    seqlen1 = np.asarray(seqlen1)
    seqlen2 = np.asarray(seqlen2)

    in_maps = []
    for c in range(NC):
        sl = slice(c * BC, (c + 1) * BC)
        t1, t2 = input1[sl], input2[sl]
        s1, s2 = seqlen1[sl], seqlen2[sl]
        stack1 = np.concatenate([t1, t2], 0)   # [128, 60] tokens, slot1
        lf = np.concatenate([s1, s2], 0)       # len of first-arg seq per row
        ls = np.concatenate([s2, s1], 0)       # len of second-arg seq per row

        m = {}
        m["x1"] = E16[stack1]                  # [128, 60, 300] row-major f16
        m["sl"] = np.stack([lf, ls - 1], axis=1).astype(np.float32)
        m["wsh"] = wflat[c * SH:(c + 1) * SH]
        in_maps.append(m)
    return in_maps


_last_exec_ns = None


def kernel(__trace=False, **inputs):
    global _last_exec_ns
    from concourse.bass_utils import run_bass_kernel_spmd

    if "nc" not in _cache:
        _cache["nc"] = _build_nc()
    nc = _cache["nc"]
    in_maps = _prep_inputs(**inputs)
    res = run_bass_kernel_spmd(nc, in_maps, core_ids=list(range(NC)),
                               trace=__trace)
    if getattr(res, "exec_time_ns", None):
        _last_exec_ns = res.exec_time_ns
    out = np.concatenate([r["out"] for r in res.results], axis=0)
    return out.astype(np.float32)


# revision 12
# speedup vs baseline: 2.4669x; 1.3505x over previous
"""Trainium2 Bass kernel for the AttentionModel (word-by-word attention entailment model).

Contract: kernel(**inputs) takes FULL unsharded inputs (as produced by
setup_inputs()) and returns the FULL [512, 2] output. Internally the batch is
sharded over 8 NeuronCores (64 sequences each); the two symmetric branches are
stacked on the partition axis so each core processes 128 "rows"
(row r < 64 -> branch1 seq r, row r >= 64 -> branch2 seq r-64).

The end-to-end call on this axon-tunneled setup is dominated by (a) host->
device transfer at ~40-50 MB/s and (b) per-instruction device overhead, so the
design minimizes both payload bytes and instruction count:
  * Only slot-1 embeddings are sent, row-major f16 [128, L, 300] per core.
    Slot 2's stack ([x2;x1]) is the same data with the row axis rotated by 64,
    derived on device. Dims-major tiles for the PE come from DMA-engine (xbar)
    transposes plus one PE transpose for the 44-row tail chunk.
  * All weights live in one flat f16 buffer; each core uploads 1/8 and the
    full buffer is reassembled on device with an AllGather.
  * Gate columns are pre-permuted to [j,i,f,o] with the LSTM forget bias baked
    into the bias row, so the three sigmoids run as ONE activation.
  * Freeze masks are uint8 + copy_predicated (1 inst instead of sub+fma).
  * The attention keeps M row-major [row, l, h]: score = reduce(M*w) needs 3
    instructions instead of a 65-instruction PSUM-chunk + DRAM-bounce pipeline.
  * alpha-weighted sums use single big tensor_reduce ops, not add-trees.

Pipeline per core: two 60-step LSTMs (with inline Y1@W_y), 60-step attention
scan, final head tanh(r_L@Wp_a + h2@Wxa) summed over branches, @U + b_out.
"""

import json

import numpy as np


def _split_multi_waits(raw: bytes) -> bytes:
    """Walrus codegen in this toolchain only encodes one sync-wait per
    instruction. Split every instruction carrying N>1 waits into N-1
    standalone EventSemaphore waits (same engine, program order) followed by
    the original instruction keeping a single wait. Sem conditions are
    monotonic, so a sequential wait chain is equivalent to the combined wait.
    """
    j = json.loads(raw)
    uid = [0]
    for fn in j.get("functions", []):
        for blk in fn.get("blocks", []):
            insts = blk.get("instructions", [])
            out = []
            for inst in insts:
                si = inst.get("sync_info")
                waits = (si or {}).get("on_wait") or []
                if len(waits) > 1:
                    eng = inst.get("engine")
                    for w in waits[:-1]:
                        uid[0] += 1
                        out.append({
                            "debug": inst.get("debug", 0),
                            "engine": eng,
                            "ins": [],
                            "outs": [],
                            "name": f"WSPLIT-{uid[0]}",
                            "opcode": "EventSemaphore",
                            "sync_info": {"on_update": [], "on_wait": [w]},
                        })
                    si["on_wait"] = [waits[-1]]
                out.append(inst)
            blk["instructions"] = out
    return json.dumps(j).encode()


def _apply_wait_split(nc):
    import concourse.bass as bass

    patched = _split_multi_waits(bass.Bass.to_json_bytes(nc))
    nc.to_json_bytes = lambda: patched
    return nc


B, L, D, H, V = 512, 60, 300, 256, 50000
NC = 8                 # cores
BC = B // NC           # 64 sequences per core
R = 2 * BC             # 128 rows (2 branches)
H4 = 4 * H             # 1024
DB = 45                # third d-chunk: rows 256..299 + bias ones-row at 44
LP = 64                # l padded to 64 for the alpha broadcast
NEG = -10000.0

# flat weight buffer layout (f16 elems); uploaded sharded + AllGathered
_WSPECS = [
    ("Wx1A", (128, 2, H4)), ("Wx2A", (128, 2, H4)),
    ("Wx1B", (DB, H4)), ("Wx2B", (DB, H4)),
    ("Wh1", (128, 2, H4)), ("Wh2", (128, 2, H4)),
    ("Wy", (128, 2, H)), ("Wha", (128, 2, H)), ("Wra", (128, 2, H)),
    ("Wta", (128, 2, H)), ("Wpa", (128, 2, H)), ("Wxa", (128, 2, H)),
    ("U", (128, 2, 2)), ("bout", (1, 2)), ("wrow", (1, H)),
]
_WOFF = {}
_off = 0
for _nm, _shp in _WSPECS:
    _WOFF[_nm] = _off
    _n = 1
    for _d in _shp:
        _n *= _d
    _off += _n
SH = 192000            # per-core weight shard elems
SW = SH * NC           # padded flat weight buffer elems
assert _off <= SW

_cache = {}


def _build_nc(l_lstm=L, l_attn=L):
    import concourse.bass as bass
    import concourse.mybir as mybir
    import concourse.tile as tile
    from concourse.masks import make_identity

    f32 = mybir.dt.float32
    f16 = mybir.dt.float16
    u8 = mybir.dt.uint8
    Alu = mybir.AluOpType
    Act = mybir.ActivationFunctionType

    nc = bass.Bass()

    # ---------------- DRAM I/O ----------------
    x1_d = nc.dram_tensor("x1", [R, L, D], f16, kind="ExternalInput")
    wsh_d = nc.dram_tensor("wsh", [SH], f16, kind="ExternalInput")
    sl_d = nc.dram_tensor("sl", [R, 2], f32, kind="ExternalInput")
    out_d = nc.dram_tensor("out", [BC, 2], f32, kind="ExternalOutput")

    with tile.TileContext(nc) as tc:
        with (
            tc.tile_pool(name="persist", bufs=1) as pp,
            tc.tile_pool(name="dram", bufs=1, space="DRAM") as dp,
        ):
            # ---- weight shard upload + AllGather into the full flat buffer
            wshard = dp.tile([SH], f16)
            wfull = dp.tile([SW], f16)
            nc.gpsimd.dma_start(wshard[:], wsh_d[:])
            nc.gpsimd.collective_compute(
                "AllGather", mybir.AluOpType.bypass,
                replica_groups=[list(range(NC))],
                ins=[wshard[:].opt()], outs=[wfull[:].opt()])

            def wslice(name):
                off = _WOFF[name]
                shp = dict(_WSPECS)[name]
                n = 1
                for d_ in shp:
                    n *= d_
                ap = wfull[off:off + n]
                if len(shp) == 2:
                    return ap.rearrange("(p n) -> p n", p=shp[0])
                return ap.rearrange("(p k n) -> p k n", p=shp[0], k=shp[1])

            # persistent sbuf tiles
            Y2T = pp.tile([128, 2, L, R], f16)    # slot2 h-state transposed, per t
            Yrh = pp.tile([128, H, LP], f16)      # slot1 h row-major [row, h, l]
            WyY = pp.tile([128, L, H], f16)       # Y1 @ W_y row-major [row, l, h]
            MM = pp.tile([128, L, H], f16)        # attention M buffer
            wWy = pp.tile([128, 2, H], f16)
            wWha = pp.tile([128, 2, H], f16)
            wWra = pp.tile([128, 2, H], f16)
            wWta = pp.tile([128, 2, H], f16)
            wWpa = pp.tile([128, 2, H], f16)
            wWxa = pp.tile([128, 2, H], f16)
            wU = pp.tile([128, 2, 2], f16)
            wbout = pp.tile([1, 2], f16)
            wones = pp.tile([1, BC], f16)
            wones1 = pp.tile([1, 128], f16)
            wrow = pp.tile([128, H], f16)         # w_a replicated on partitions
            sl_sb = pp.tile([R, 2], f32)
            lio = pp.tile([R, LP], f32)
            maskadd = pp.tile([R, LP], f16)
            sel = pp.tile([R, LP], f32)
            mfu = {1: pp.tile([R, LP], u8, name="mfu1"),
                   2: pp.tile([R, LP], u8, name="mfu2")}
            ident = pp.tile([128, 128], f32)
            ident16 = pp.tile([128, 128], f16)
            # states
            rr16 = pp.tile([R, H], f16)           # r (row major)
            rT = pp.tile([128, 2, R], f16)        # r transposed
            rL = pp.tile([R, H], f32)
            uu = pp.tile([R, H], f32)
            TT = pp.tile([R, H], f32)

            make_identity(nc, ident[:])
            make_identity(nc, ident16[:])
            for t_ in (Yrh, rT):
                nc.vector.memset(t_[:], 0.0)
            nc.vector.memset(rL[:], 0.0)
            nc.vector.memset(wones[:], 1.0)
            nc.vector.memset(wones1[:], 1.0)

            for dst, nm in [
                (wWy, "Wy"), (wWha, "Wha"), (wWra, "Wra"), (wWta, "Wta"),
                (wWpa, "Wpa"), (wWxa, "Wxa"), (wU, "U"), (wbout, "bout"),
            ]:
                nc.sync.dma_start(dst[:], wslice(nm))

            # ---- w_a replicated across partitions via ones-matmul
            with tc.tile_pool(name="init_ps", bufs=1, space="PSUM") as ips:
                wr_sb = pp.tile([1, H], f16)
                nc.sync.dma_start(wr_sb[:], wslice("wrow"))
                wp = ips.tile([128, H], f32, tag="wp")
                nc.tensor.matmul(wp[:], wones1[:], wr_sb[:], start=True, stop=True)
                nc.scalar.copy(wrow[:], wp[:])

            # ---- masks from seqlens: lf = sl[:,0], ls-1 = sl[:,1]
            nc.sync.dma_start(sl_sb[:], sl_d[:])
            nc.gpsimd.iota(lio[:], pattern=[[1, LP]], base=0,
                           channel_multiplier=0,
                           allow_small_or_imprecise_dtypes=True)
            nc.vector.tensor_scalar(
                mfu[1][:], lio[:], sl_sb[:, 0:1], None, op0=Alu.is_lt)
            nc.vector.tensor_scalar(
                mfu[2][:], lio[:], sl_sb[:, 1:2], None, op0=Alu.is_le)
            nc.vector.tensor_scalar(
                maskadd[:], lio[:], sl_sb[:, 0:1], NEG,
                op0=Alu.is_ge, op1=Alu.mult)
            nc.vector.tensor_scalar(
                sel[:], lio[:], sl_sb[:, 1:2], None, op0=Alu.is_equal)

            # ======== Phase 1: the two LSTMs (+ inline Y1 @ W_y) ========
            with (
                tc.tile_pool(name="lstm", bufs=1) as lp,
                tc.tile_pool(name="lstm_xt", bufs=2) as lxt,
                tc.tile_pool(name="lstm_ps", bufs=2, space="PSUM") as lps,
                tc.tile_pool(name="xtr_ps", bufs=2, space="PSUM") as xps,
                tc.tile_pool(name="wyy_ps", bufs=2, space="PSUM") as wps,
            ):
                wWx1A = lp.tile([128, 2, H4], f16, name="wx1a")
                wWx2A = lp.tile([128, 2, H4], f16, name="wx2a")
                wWx1B = lp.tile([DB, H4], f16, name="wx1b")
                wWx2B = lp.tile([DB, H4], f16, name="wx2b")
                wWh1 = lp.tile([128, 2, H4], f16, name="wh1")
                wWh2 = lp.tile([128, 2, H4], f16, name="wh2")
                for dst, nm in [(wWx1A, "Wx1A"), (wWx2A, "Wx2A"),
                                (wWx1B, "Wx1B"), (wWx2B, "Wx2B"),
                                (wWh1, "Wh1"), (wWh2, "Wh2")]:
                    nc.sync.dma_start(dst[:], wslice(nm))

                cc = {1: lp.tile([R, H], f32, name="c1"),
                      2: lp.tile([R, H], f32, name="c2")}
                hh = {1: lp.tile([R, H], f16, name="h1"),
                      2: lp.tile([R, H], f16, name="h2")}
                for s in (1, 2):
                    nc.vector.memset(cc[s][:], 0.0)
                    nc.vector.memset(hh[s][:], 0.0)

                wWxA = {1: wWx1A, 2: wWx2A}
                wWxB = {1: wWx1B, 2: wWx2B}
                wWh = {1: wWh1, 2: wWh2}

                # pre-set the bias ones-row (44) in both xb1 pool buffers;
                # per-step writes only touch rows 0:44 so it persists, and
                # xb2's rotated copy carries it over
                for _ in range(2):
                    b_ = lxt.tile([DB, R], f16, tag="xb1")
                    nc.vector.memset(b_[:], 1.0)

                prev_hT1 = None
                for t in range(l_lstm):
                    # slot-1 x_t: xbar-transpose the two 128-row d-chunks,
                    # PE-transpose the 44-row tail
                    xt1 = lxt.tile([128, 2, R], f16, tag="xt1")
                    xb1 = lxt.tile([DB, R], f16, tag="xb1")
                    nc.sync.dma_start_transpose(xt1[:, 0, :], x1_d[:, t, 0:128])
                    nc.sync.dma_start_transpose(xt1[:, 1, :], x1_d[:, t, 128:256])
                    xrb = lxt.tile([R, 44], f16, tag="xrb")
                    nc.gpsimd.dma_start(xrb[:], x1_d[:, t, 256:300])
                    tpx = xps.tile([128, 128], f16, tag="xtp")
                    nc.tensor.transpose(tpx[0:44, :], xrb[:], ident16[:])
                    nc.scalar.copy(xb1[0:44, :], tpx[0:44, :])
                    # slot-2 x_t = slot-1 rotated by 64 on the row axis
                    xt2 = lxt.tile([128, 2, R], f16, tag="xt2")
                    xb2 = lxt.tile([DB, R], f16, tag="xb2")
                    nc.vector.tensor_copy(xt2[:, :, 0:BC], xt1[:, :, BC:R])
                    nc.vector.tensor_copy(xt2[:, :, BC:R], xt1[:, :, 0:BC])
                    nc.gpsimd.tensor_copy(xb2[:, 0:BC], xb1[:, BC:R])
                    nc.gpsimd.tensor_copy(xb2[:, BC:R], xb1[:, 0:BC])
                    xts = {1: xt1, 2: xt2}
                    xbs = {1: xb1, 2: xb2}
                    hT1 = lxt.tile([128, 2, R], f16, tag="hT1")
                    for s in (1, 2):
                        gps = lps.tile([R, H4], f32, tag="gates")
                        for nck in range(2):
                            nsl = slice(nck * 512, (nck + 1) * 512)
                            mms = [(xts[s][:, 0, :], wWxA[s][:, 0, nsl]),
                                   (xts[s][:, 1, :], wWxA[s][:, 1, nsl]),
                                   (xbs[s][:, :], wWxB[s][:, nsl])]
                            if t > 0:
                                hTs = (prev_hT1[:, kt_, :] for kt_ in range(2)) \
                                    if s == 1 else \
                                    (Y2T[:, kt_, t - 1, :] for kt_ in range(2))
                                mms += [(hT, wWh[s][:, kt_, nsl])
                                        for kt_, hT in enumerate(hTs)]
                            for i, (a_, b_) in enumerate(mms):
                                nc.tensor.matmul(
                                    gps[:, nsl], a_, b_,
                                    start=(i == 0), stop=(i == len(mms) - 1))
                        # gates pre-permuted to [j, i, f, o]; f bias baked
                        tj = lp.tile([R, H], f32, tag="tj")
                        sio = lp.tile([R, 3 * H], f32, tag="sio")
                        nc.scalar.activation(tj[:], gps[:, 0:256], Act.Tanh)
                        nc.scalar.activation(sio[:], gps[:, 256:1024], Act.Sigmoid)
                        t1 = lp.tile([R, H], f32, tag="t1")
                        t2 = lp.tile([R, H], f32, tag="t2")
                        cn = lp.tile([R, H], f32, tag="cn")
                        nc.vector.tensor_tensor(
                            t1[:], cc[s][:], sio[:, 256:512], op=Alu.mult)
                        nc.gpsimd.tensor_tensor(
                            t2[:], tj[:], sio[:, 0:256], op=Alu.mult)
                        nc.vector.tensor_tensor(cn[:], t1[:], t2[:], op=Alu.add)
                        # freeze c
                        nc.vector.copy_predicated(
                            cc[s][:], mfu[s][:, t:t + 1].broadcast_to([R, H]), cn[:])
                        tcn = lp.tile([R, H], f32, tag="tcn")
                        nc.scalar.activation(tcn[:], cn[:], Act.Tanh)
                        hn = lp.tile([R, H], f16, tag="hn")
                        nc.vector.tensor_tensor(
                            hn[:], tcn[:], sio[:, 512:768], op=Alu.mult)
                        # freeze h
                        nc.vector.copy_predicated(
                            hh[s][:], mfu[s][:, t:t + 1].broadcast_to([R, H]), hn[:])
                        # transpose frozen h via xbar DMA
                        if s == 1:
                            nc.sync.dma_start_transpose(hT1[:, 0, :], hh[1][:, 0:128])
                            nc.sync.dma_start_transpose(hT1[:, 1, :], hh[1][:, 128:256])
                            nc.gpsimd.tensor_copy(Yrh[:, :, t], hh[1][:])
                            # inline WyY[:, t, :] = Y1_t @ W_y
                            wyp = wps.tile([R, H], f32, tag="wyy")
                            for kt in range(2):
                                nc.tensor.matmul(
                                    wyp[:], hT1[:, kt, :], wWy[:, kt, :],
                                    start=(kt == 0), stop=(kt == 1))
                            if t % 2 == 0:
                                nc.scalar.copy(WyY[:, t, :], wyp[:])
                            else:
                                nc.vector.tensor_copy(WyY[:, t, :], wyp[:])
                        else:
                            nc.sync.dma_start_transpose(
                                Y2T[:, 0, t, :], hh[2][:, 0:128])
                            nc.sync.dma_start_transpose(
                                Y2T[:, 1, t, :], hh[2][:, 128:256])
                    prev_hT1 = hT1

            # ======== Phase 3: attention scan ========
            with (
                tc.tile_pool(name="attn", bufs=1) as ap,
                tc.tile_pool(name="ptree", bufs=1) as ptp,
                tc.tile_pool(name="at_ps", bufs=1, space="PSUM") as aps,
            ):
                e64 = ap.tile([R, LP], f16)
                nc.vector.memset(e64[:], 0.0)
                den = ap.tile([R, 1], f32)
                rden = ap.tile([R, 1], f32)
                al = ap.tile([R, LP], f16)
                s_rl = ap.tile([R, L], f32)
                sm = ap.tile([R, L], f32)
                tmps = ap.tile([R, H], f16)
                LH = L // 2

                for t in range(l_attn):
                    # tmp = h2_t @ Wha + r @ Wra   (row-major [row, h], PSUM)
                    tmpps = aps.tile([R, H], f32, tag="tmps")
                    for kt in range(2):
                        nc.tensor.matmul(
                            tmpps[:], Y2T[:, kt, t, :], wWha[:, kt, :],
                            start=(kt == 0), stop=False)
                    for kt in range(2):
                        nc.tensor.matmul(
                            tmpps[:], rT[:, kt, :], wWra[:, kt, :],
                            start=False, stop=(kt == 1))
                    # rwt = r @ Wta ; T = tanh
                    rwt = aps.tile([R, H], f32, tag="rwt")
                    for kt in range(2):
                        nc.tensor.matmul(
                            rwt[:], rT[:, kt, :], wWta[:, kt, :],
                            start=(kt == 0), stop=(kt == 1))
                    nc.scalar.activation(TT[:], rwt[:], Act.Tanh)
                    # M = WyY + tmp (broadcast over l), split l-halves v/g
                    nc.scalar.copy(tmps[:], tmpps[:])
                    nc.vector.tensor_tensor(
                        MM[:, 0:LH, :], WyY[:, 0:LH, :],
                        tmps[:].unsqueeze(1).broadcast_to([R, LH, H]),
                        op=Alu.add)
                    nc.gpsimd.tensor_tensor(
                        MM[:, LH:L, :], WyY[:, LH:L, :],
                        tmps[:].unsqueeze(1).broadcast_to([R, LH, H]),
                        op=Alu.add)
                    mflat = MM[:].rearrange("p l h -> p (l h)")
                    nc.scalar.activation(mflat[:], mflat[:], Act.Tanh)
                    # score[r, l] = reduce_h(M * w)
                    nc.vector.tensor_tensor(
                        MM[:, 0:LH, :], MM[:, 0:LH, :],
                        wrow[:].unsqueeze(1).broadcast_to([R, LH, H]),
                        op=Alu.mult)
                    nc.gpsimd.tensor_tensor(
                        MM[:, LH:L, :], MM[:, LH:L, :],
                        wrow[:].unsqueeze(1).broadcast_to([R, LH, H]),
                        op=Alu.mult)
                    nc.vector.tensor_reduce(
                        s_rl[:], MM[:], axis=mybir.AxisListType.X, op=Alu.add)
                    # masked softmax -> alpha
                    nc.vector.tensor_tensor(
                        sm[:], s_rl[:], maskadd[:, 0:L], op=Alu.add)
                    nc.scalar.activation(
                        e64[:, 0:L], sm[:], Act.Exp, accum_out=den[:])
                    nc.vector.reciprocal(rden[:], den[:])
                    nc.vector.tensor_scalar_mul(al[:], e64[:], rden[:])
                    # u = sum_l alpha * Y : split h-halves v/g + big reduces
                    P0 = ptp.tile([128, 128, LP], f16, tag="P0")
                    P1 = ptp.tile([128, 128, LP], f16, tag="P1")
                    nc.vector.tensor_tensor(
                        P0[:], Yrh[:, 0:128, :],
                        al[:].unsqueeze(1).broadcast_to([R, 128, LP]),
                        op=Alu.mult)
                    nc.gpsimd.tensor_tensor(
                        P1[:], Yrh[:, 128:256, :],
                        al[:].unsqueeze(1).broadcast_to([R, 128, LP]),
                        op=Alu.mult)
                    nc.vector.tensor_reduce(
                        uu[:, 0:128], P0[:], axis=mybir.AxisListType.X, op=Alu.add)
                    nc.vector.tensor_reduce(
                        uu[:, 128:256], P1[:], axis=mybir.AxisListType.X, op=Alu.add)
                    # r = u + T ; r_L += sel_t * r ; transpose r via xbar
                    nc.vector.tensor_tensor(rr16[:], uu[:], TT[:], op=Alu.add)
                    nc.vector.scalar_tensor_tensor(
                        rL[:], rr16[:], sel[:, t:t + 1], rL[:],
                        op0=Alu.mult, op1=Alu.add)
                    nc.sync.dma_start_transpose(rT[:, 0, :], rr16[:, 0:128])
                    nc.sync.dma_start_transpose(rT[:, 1, :], rr16[:, 128:256])

                # ======== Phase 4: final head ========
                rLT = ap.tile([128, 2, R], f16)
                for kt in range(2):
                    tp = aps.tile([128, 128], f32, tag="rtp")
                    nc.tensor.transpose(
                        tp[:], rL[:, kt * 128:(kt + 1) * 128], ident[:])
                    nc.scalar.copy(rLT[:, kt, :], tp[:])
                fT = ap.tile([128, 2, R], f16)
                for mt in range(2):
                    msl = slice(mt * 128, (mt + 1) * 128)
                    fps = aps.tile([128, R], f32, tag="fps")
                    for kt in range(2):
                        nc.tensor.matmul(
                            fps[:], wWpa[:, kt, msl], rLT[:, kt, :],
                            start=(kt == 0), stop=False)
                    for kt in range(2):
                        nc.tensor.matmul(
                            fps[:], wWxa[:, kt, msl], Y2T[:, kt, L - 1, :],
                            start=False, stop=(kt == 1))
                    nc.scalar.activation(fT[:, mt, :], fps[:], Act.Tanh)
                lhT = ap.tile([128, 2, BC], f16)
                nc.vector.tensor_tensor(
                    lhT[:], fT[:, :, 0:BC], fT[:, :, BC:R], op=Alu.add)
                ops_ = aps.tile([BC, 2], f32, tag="ops")
                for kt in range(2):
                    nc.tensor.matmul(
                        ops_[:], lhT[:, kt, :], wU[:, kt, :],
                        start=(kt == 0), stop=False)
                nc.tensor.matmul(ops_[:], wones[:], wbout[:], start=False, stop=True)
                osb = ap.tile([BC, 2], f32)
                nc.vector.tensor_copy(osb[:], ops_[:])
                nc.sync.dma_start(out_d[:], osb[:])

    return _apply_wait_split(nc)


# gate-column permutation: TF order [i,j,f,o] -> device order [j,i,f,o]
_GPERM = np.concatenate([
    np.arange(256, 512), np.arange(0, 256),
    np.arange(512, 768), np.arange(768, 1024)])


def _prep_inputs(E, Wx1, Wh1, b1, Wx2, Wh2, b2, W_y, Wh_a, Wr_a, w_a, Wt_a,
                 Wp_a, Wxa, U, b_out, input1, input2, seqlen1, seqlen2):
    """Build the per-core input maps (host-side sharding + packing)."""
    f16 = np.float16
    E16 = np.asarray(E, np.float32).astype(f16)

    def pack_w2(W, perm=None):
        W = np.asarray(W, np.float32)
        if perm is not None:
            W = W[:, perm]
        return np.stack([W[0:128], W[128:256]], axis=1).astype(f16)

    def packB(W, b):
        W = np.asarray(W, np.float32)[:, _GPERM]
        b = np.asarray(b, np.float32)[_GPERM].copy()
        b[512:768] += 1.0  # TF forget_bias baked into the bias row
        out = np.zeros((DB, H4), np.float32)
        out[0:44] = W[256:300]
        out[44] = b
        return out.astype(f16)

    parts = [
        pack_w2(Wx1, _GPERM).ravel(), pack_w2(Wx2, _GPERM).ravel(),
        packB(Wx1, b1).ravel(), packB(Wx2, b2).ravel(),
        pack_w2(Wh1, _GPERM).ravel(), pack_w2(Wh2, _GPERM).ravel(),
        pack_w2(W_y).ravel(), pack_w2(Wh_a).ravel(), pack_w2(Wr_a).ravel(),
        pack_w2(Wt_a).ravel(), pack_w2(Wp_a).ravel(), pack_w2(Wxa).ravel(),
        pack_w2(U).ravel(),
        np.asarray(b_out, np.float32).reshape(1, 2).astype(f16).ravel(),
        np.asarray(w_a, np.float32).reshape(1, H).astype(f16).ravel(),
    ]
    wflat = np.concatenate(parts)
    assert wflat.size == _off
    wflat = np.concatenate([wflat, np.zeros(SW - wflat.size, f16)])

    input1 = np.asarray(input1)
    input2 = np.asarray(input2)
    seqlen1 = np.asarray(seqlen1)
    seqlen2 = np.asarray(seqlen2)

    in_maps = []
    for c in range(NC):
        sl = slice(c * BC, (c + 1) * BC)
        t1, t2 = input1[sl], input2[sl]
        s1, s2 = seqlen1[sl], seqlen2[sl]
        stack1 = np.concatenate([t1, t2], 0)   # [128, 60] tokens, slot1
        lf = np.concatenate([s1, s2], 0)       # len of first-arg seq per row
        ls = np.concatenate([s2, s1], 0)       # len of second-arg seq per row

        m = {}
        m["x1"] = E16[stack1]                  # [128, 60, 300] row-major f16
        m["sl"] = np.stack([lf, ls - 1], axis=1).astype(np.float32)
        m["wsh"] = wflat[c * SH:(c + 1) * SH]
        in_maps.append(m)
    return in_maps


_last_exec_ns = None


def kernel(__trace=False, **inputs):
    global _last_exec_ns
    from concourse.bass_utils import run_bass_kernel_spmd

    if "nc" not in _cache:
        _cache["nc"] = _build_nc()
    nc = _cache["nc"]
    in_maps = _prep_inputs(**inputs)
    res = run_bass_kernel_spmd(nc, in_maps, core_ids=list(range(NC)),
                               trace=__trace)
    if getattr(res, "exec_time_ns", None):
        _last_exec_ns = res.exec_time_ns
    out = np.concatenate([r["out"] for r in res.results], axis=0)
    return out.astype(np.float32)


# revision 17
# speedup vs baseline: 2.8954x; 1.1737x over previous
"""Trainium2 Bass kernel for the AttentionModel (word-by-word attention entailment model).

Contract: kernel(**inputs) takes FULL unsharded inputs (as produced by
setup_inputs()) and returns the FULL [512, 2] output. Internally the batch is
sharded over 8 NeuronCores (64 sequences each); the two symmetric branches are
stacked on the partition axis so each core processes 128 "rows"
(row r < 64 -> branch1 seq r, row r >= 64 -> branch2 seq r-64).

The end-to-end call on this axon-tunneled setup is dominated by (a) host->
device transfer at ~40-50 MB/s and (b) per-instruction device overhead, so the
design minimizes both payload bytes and instruction count:
  * Only slot-1 embeddings are sent, row-major INT8 [128, L, 300] per core
    (2.3 MB): E is symmetrically quantized at 4.5 sigma and the quant scale is
    folded into the Wx weights host-side; the device just int8->f16 converts.
    Slot 2's stack ([x2;x1]) is the same data with the row axis rotated by 64,
    derived on device. Dims-major tiles for the PE come from DMA-engine (xbar)
    transposes plus one PE transpose for the 44-row tail chunk.
  * All weights live in one flat f16 buffer; each core uploads 1/8 and the
    full buffer is reassembled on device with an AllGather.
  * Gate columns are pre-permuted to [j,i,f,o] with the LSTM forget bias baked
    into the bias row, so the three sigmoids run as ONE activation; gates for
    BOTH LSTM slots accumulate in one f16 PSUM tile [128, 2, 1024] so every
    elementwise/activation op handles both slots in a single instruction.
  * Freeze masks are uint8 + copy_predicated.
  * The attention keeps M row-major [row, l, h]: score = reduce(M*w) is 3 big
    instructions instead of a 65-instruction PSUM-chunk + DRAM-bounce
    pipeline; Wr_a|Wt_a are concatenated so r@Wra and r@Wta share matmuls.

Pipeline per core: two 60-step LSTMs (with inline Y1@W_y), 60-step attention
scan, final head tanh(r_L@Wp_a + h2@Wxa) summed over branches, @U + b_out.
"""

import json

import numpy as np


def _split_multi_waits(raw: bytes) -> bytes:
    """Walrus codegen in this toolchain only encodes one sync-wait per
    instruction. Split every instruction carrying N>1 waits into N-1
    standalone EventSemaphore waits (same engine, program order) followed by
    the original instruction keeping a single wait. Sem conditions are
    monotonic, so a sequential wait chain is equivalent to the combined wait.
    """
    j = json.loads(raw)
    uid = [0]
    for fn in j.get("functions", []):
        for blk in fn.get("blocks", []):
            insts = blk.get("instructions", [])
            out = []
            for inst in insts:
                si = inst.get("sync_info")
                waits = (si or {}).get("on_wait") or []
                if len(waits) > 1:
                    eng = inst.get("engine")
                    for w in waits[:-1]:
                        uid[0] += 1
                        out.append({
                            "debug": inst.get("debug", 0),
                            "engine": eng,
                            "ins": [],
                            "outs": [],
                            "name": f"WSPLIT-{uid[0]}",
                            "opcode": "EventSemaphore",
                            "sync_info": {"on_update": [], "on_wait": [w]},
                        })
                    si["on_wait"] = [waits[-1]]
                out.append(inst)
            blk["instructions"] = out
    return json.dumps(j).encode()


def _apply_wait_split(nc):
    import concourse.bass as bass

    patched = _split_multi_waits(bass.Bass.to_json_bytes(nc))
    nc.to_json_bytes = lambda: patched
    return nc


B, L, D, H, V = 512, 60, 300, 256, 50000
NC = 8                 # cores
BC = B // NC           # 64 sequences per core
R = 2 * BC             # 128 rows (2 branches)
H4 = 4 * H             # 1024
DB = 45                # third d-chunk: rows 256..299 + bias ones-row at 44
LP = 64                # l padded to 64 for the alpha broadcast
NEG = -10000.0
NSIG = 4.5             # int8 clip point for the embedding quantization

# flat weight buffer layout (f16 elems); uploaded sharded + AllGathered
_WSPECS = [
    ("Wx1A", (128, 2, H4)), ("Wx2A", (128, 2, H4)),
    ("Wx1B", (DB, H4)), ("Wx2B", (DB, H4)),
    ("Wh1", (128, 2, H4)), ("Wh2", (128, 2, H4)),
    ("Wy", (128, 2, H)), ("Wha", (128, 2, H)), ("Wrta", (128, 2, 2 * H)),
    ("Wpa", (128, 2, H)), ("Wxa", (128, 2, H)),
    ("U", (128, 2, 2)), ("bout", (1, 2)), ("wrow", (1, H)),
]
_WOFF = {}
_off = 0
for _nm, _shp in _WSPECS:
    _WOFF[_nm] = _off
    _n = 1
    for _d in _shp:
        _n *= _d
    _off += _n
SH = 192000            # per-core weight shard elems
SW = SH * NC           # padded flat weight buffer elems
assert _off <= SW

_cache = {}


def _build_nc(l_lstm=L, l_attn=L):
    import concourse.bass as bass
    import concourse.mybir as mybir
    import concourse.tile as tile
    from concourse.masks import make_identity

    f32 = mybir.dt.float32
    f16 = mybir.dt.float16
    u8 = mybir.dt.uint8
    i8 = mybir.dt.int8
    Alu = mybir.AluOpType
    Act = mybir.ActivationFunctionType

    nc = bass.Bass()

    # ---------------- DRAM I/O ----------------
    x1_d = nc.dram_tensor("x1", [R, L, D], i8, kind="ExternalInput")
    wsh_d = nc.dram_tensor("wsh", [SH], f16, kind="ExternalInput")
    sl_d = nc.dram_tensor("sl", [R, 2], f32, kind="ExternalInput")
    out_d = nc.dram_tensor("out", [BC, 2], f32, kind="ExternalOutput")

    with tile.TileContext(nc) as tc:
        with (
            tc.tile_pool(name="persist", bufs=1) as pp,
            tc.tile_pool(name="dram", bufs=1, space="DRAM") as dp,
        ):
            # ---- weight shard upload + AllGather into the full flat buffer
            wshard = dp.tile([SH], f16)
            wfull = dp.tile([SW], f16)
            nc.gpsimd.dma_start(wshard[:], wsh_d[:])
            nc.gpsimd.collective_compute(
                "AllGather", mybir.AluOpType.bypass,
                replica_groups=[list(range(NC))],
                ins=[wshard[:].opt()], outs=[wfull[:].opt()])

            def wslice(name):
                off = _WOFF[name]
                shp = dict(_WSPECS)[name]
                n = 1
                for d_ in shp:
                    n *= d_
                ap = wfull[off:off + n]
                if len(shp) == 2:
                    return ap.rearrange("(p n) -> p n", p=shp[0])
                return ap.rearrange("(p k n) -> p k n", p=shp[0], k=shp[1])

            # persistent sbuf tiles
            Y2T = pp.tile([128, 2, L, R], f16)    # slot2 h-state transposed, per t
            Yrh = pp.tile([128, H, LP], f16)      # slot1 h row-major [row, h, l]
            WyY = pp.tile([128, L, H], f16)       # Y1 @ W_y row-major [row, l, h]
            MM = pp.tile([128, L, H], f16)        # attention M buffer
            wWy = pp.tile([128, 2, H], f16)
            wWha = pp.tile([128, 2, H], f16)
            wWrta = pp.tile([128, 2, 2 * H], f16)
            wWpa = pp.tile([128, 2, H], f16)
            wWxa = pp.tile([128, 2, H], f16)
            wU = pp.tile([128, 2, 2], f16)
            wbout = pp.tile([1, 2], f16)
            wones = pp.tile([1, BC], f16)
            wones1 = pp.tile([1, 128], f16)
            wrow = pp.tile([128, H], f16)         # w_a replicated on partitions
            sl_sb = pp.tile([R, 2], f32)
            lio = pp.tile([R, LP], f32)
            maskadd = pp.tile([R, LP], f16)
            sel = pp.tile([R, LP], f32)
            mfu = pp.tile([R, 2, LP], u8)         # freeze masks, both slots
            ident = pp.tile([128, 128], f32)
            ident16 = pp.tile([128, 128], f16)
            # states
            rr16 = pp.tile([R, H], f16)           # r (row major)
            rT = pp.tile([128, 2, R], f16)        # r transposed
            rL = pp.tile([R, H], f32)
            uu = pp.tile([R, H], f32)
            TT = pp.tile([R, H], f32)

            make_identity(nc, ident[:])
            make_identity(nc, ident16[:])
            for t_ in (Yrh, rT):
                nc.vector.memset(t_[:], 0.0)
            nc.vector.memset(rL[:], 0.0)
            nc.vector.memset(wones[:], 1.0)
            nc.vector.memset(wones1[:], 1.0)

            for dst, nm in [
                (wWy, "Wy"), (wWha, "Wha"), (wWrta, "Wrta"),
                (wWpa, "Wpa"), (wWxa, "Wxa"), (wU, "U"), (wbout, "bout"),
            ]:
                nc.sync.dma_start(dst[:], wslice(nm))

            # ---- w_a replicated across partitions via ones-matmul
            with tc.tile_pool(name="init_ps", bufs=1, space="PSUM") as ips:
                wr_sb = pp.tile([1, H], f16)
                nc.sync.dma_start(wr_sb[:], wslice("wrow"))
                wp = ips.tile([128, H], f32, tag="wp")
                nc.tensor.matmul(wp[:], wones1[:], wr_sb[:], start=True, stop=True)
                nc.scalar.copy(wrow[:], wp[:])

            # ---- masks from seqlens: lf = sl[:,0], ls-1 = sl[:,1]
            nc.sync.dma_start(sl_sb[:], sl_d[:])
            nc.gpsimd.iota(lio[:], pattern=[[1, LP]], base=0,
                           channel_multiplier=0,
                           allow_small_or_imprecise_dtypes=True)
            nc.vector.tensor_scalar(
                mfu[:, 0, :], lio[:], sl_sb[:, 0:1], None, op0=Alu.is_lt)
            nc.vector.tensor_scalar(
                mfu[:, 1, :], lio[:], sl_sb[:, 1:2], None, op0=Alu.is_le)
            nc.vector.tensor_scalar(
                maskadd[:], lio[:], sl_sb[:, 0:1], NEG,
                op0=Alu.is_ge, op1=Alu.mult)
            nc.vector.tensor_scalar(
                sel[:], lio[:], sl_sb[:, 1:2], None, op0=Alu.is_equal)

            # ======== Phase 1: the two LSTMs (+ inline Y1 @ W_y) ========
            with (
                tc.tile_pool(name="lstm", bufs=1) as lp,
                tc.tile_pool(name="lstm_xq", bufs=3) as lxq,
                tc.tile_pool(name="lstm_xt", bufs=2) as lxt,
                tc.tile_pool(name="lstm_ps", bufs=1, space="PSUM") as lps,
                tc.tile_pool(name="xtr_ps", bufs=2, space="PSUM") as xps,
                tc.tile_pool(name="wyy_ps", bufs=1, space="PSUM") as wps,
            ):
                wWx1A = lp.tile([128, 2, H4], f16, name="wx1a")
                wWx2A = lp.tile([128, 2, H4], f16, name="wx2a")
                wWx1B = lp.tile([DB, H4], f16, name="wx1b")
                wWx2B = lp.tile([DB, H4], f16, name="wx2b")
                wWh1 = lp.tile([128, 2, H4], f16, name="wh1")
                wWh2 = lp.tile([128, 2, H4], f16, name="wh2")
                for dst, nm in [(wWx1A, "Wx1A"), (wWx2A, "Wx2A"),
                                (wWx1B, "Wx1B"), (wWx2B, "Wx2B"),
                                (wWh1, "Wh1"), (wWh2, "Wh2")]:
                    nc.sync.dma_start(dst[:], wslice(nm))

                wWxA = {1: wWx1A, 2: wWx2A}
                wWxB = {1: wWx1B, 2: wWx2B}
                wWh = {1: wWh1, 2: wWh2}

                cc2 = lp.tile([R, 2, H], f32, name="cc2")   # cell state, both slots
                hh2 = lp.tile([R, 2, H], f16, name="hh2")   # hidden, both slots
                nc.vector.memset(cc2[:], 0.0)
                nc.vector.memset(hh2[:], 0.0)

                # pre-set the bias ones-row (44) in both xb1 pool buffers;
                # per-step writes only touch rows 0:44 so it persists, and
                # xb2's rotated copy carries it over
                for _ in range(2):
                    b_ = lxt.tile([DB, R], f16, tag="xb1")
                    nc.vector.memset(b_[:], 1.0)

                prev_hT1 = None
                for t in range(l_lstm):
                    # slot-1 x_t: int8 load, f16 convert, xbar-transpose the
                    # two 128-row d-chunks, PE-transpose the 44-row tail
                    xq8 = lxq.tile([R, D], i8, tag="xq8")
                    nc.gpsimd.dma_start(xq8[:], x1_d[:, t, :])
                    xq16 = lxq.tile([R, D], f16, tag="xq16")
                    nc.gpsimd.tensor_copy(xq16[:], xq8[:])
                    xt1 = lxt.tile([128, 2, R], f16, tag="xt1")
                    xb1 = lxt.tile([DB, R], f16, tag="xb1")
                    nc.sync.dma_start_transpose(xt1[:, 0, :], xq16[:, 0:128])
                    nc.sync.dma_start_transpose(xt1[:, 1, :], xq16[:, 128:256])
                    tpx = xps.tile([128, 128], f16, tag="xtp")
                    nc.tensor.transpose(tpx[0:44, :], xq16[:, 256:300], ident16[:])
                    nc.scalar.copy(xb1[0:44, :], tpx[0:44, :])
                    # slot-2 x_t = slot-1 rotated by 64 on the row axis
                    xt2 = lxt.tile([128, 2, R], f16, tag="xt2")
                    xb2 = lxt.tile([DB, R], f16, tag="xb2")
                    nc.vector.tensor_copy(xt2[:, :, 0:BC], xt1[:, :, BC:R])
                    nc.vector.tensor_copy(xt2[:, :, BC:R], xt1[:, :, 0:BC])
                    nc.gpsimd.tensor_copy(xb2[:, 0:BC], xb1[:, BC:R])
                    nc.gpsimd.tensor_copy(xb2[:, BC:R], xb1[:, 0:BC])
                    xts = {1: xt1, 2: xt2}
                    xbs = {1: xb1, 2: xb2}
                    hT1 = lxt.tile([128, 2, R], f16, tag="hT1")
                    # gates for BOTH slots in one f32 psum tile [R, 2, 1024]
                    gps = lps.tile([R, 2, H4], f32, tag="gates")
                    for s in (1, 2):
                        for nck in range(2):
                            nsl = slice(nck * 512, (nck + 1) * 512)
                            mms = [(xts[s][:, 0, :], wWxA[s][:, 0, nsl]),
                                   (xts[s][:, 1, :], wWxA[s][:, 1, nsl]),
                                   (xbs[s][:, :], wWxB[s][:, nsl])]
                            if t > 0:
                                hTs = [prev_hT1[:, kt_, :] for kt_ in range(2)] \
                                    if s == 1 else \
                                    [Y2T[:, kt_, t - 1, :] for kt_ in range(2)]
                                mms += [(hT, wWh[s][:, kt_, nsl])
                                        for kt_, hT in enumerate(hTs)]
                            for i, (a_, b_) in enumerate(mms):
                                nc.tensor.matmul(
                                    gps[:, s - 1, nsl], a_, b_,
                                    start=(i == 0), stop=(i == len(mms) - 1))
                    # gates pre-permuted to [j, i, f, o]; f bias baked.
                    # process BOTH slots per instruction via [R, 2, *] APs
                    tj = lp.tile([R, 2, H], f32, tag="tj")
                    sio = lp.tile([R, 2, 3 * H], f32, tag="sio")
                    nc.scalar.activation(tj[:], gps[:, :, 0:256], Act.Tanh)
                    nc.scalar.activation(sio[:], gps[:, :, 256:1024], Act.Sigmoid)
                    t1 = lp.tile([R, 2, H], f32, tag="t1")
                    t2 = lp.tile([R, 2, H], f32, tag="t2")
                    cn = lp.tile([R, 2, H], f32, tag="cn")
                    nc.vector.tensor_tensor(
                        t1[:], cc2[:], sio[:, :, 256:512], op=Alu.mult)
                    nc.gpsimd.tensor_tensor(
                        t2[:], tj[:], sio[:, :, 0:256], op=Alu.mult)
                    nc.vector.tensor_tensor(cn[:], t1[:], t2[:], op=Alu.add)
                    nc.vector.copy_predicated(
                        cc2[:], mfu[:, :, t:t + 1].broadcast_to([R, 2, H]), cn[:])
                    tcn = lp.tile([R, 2, H], f32, tag="tcn")
                    nc.scalar.activation(tcn[:], cn[:], Act.Tanh)
                    hn = lp.tile([R, 2, H], f16, tag="hn")
                    nc.gpsimd.tensor_tensor(
                        hn[:], tcn[:], sio[:, :, 512:768], op=Alu.mult)
                    nc.vector.copy_predicated(
                        hh2[:], mfu[:, :, t:t + 1].broadcast_to([R, 2, H]), hn[:])
                    # transpose frozen h via xbar DMA
                    nc.sync.dma_start_transpose(hT1[:, 0, :], hh2[:, 0, 0:128])
                    nc.sync.dma_start_transpose(hT1[:, 1, :], hh2[:, 0, 128:256])
                    nc.sync.dma_start_transpose(Y2T[:, 0, t, :], hh2[:, 1, 0:128])
                    nc.sync.dma_start_transpose(Y2T[:, 1, t, :], hh2[:, 1, 128:256])
                    nc.gpsimd.tensor_copy(Yrh[:, :, t], hh2[:, 0, :])
                    # inline WyY[:, t, :] = Y1_t @ W_y
                    wyp = wps.tile([R, H], f32, tag="wyy")
                    for kt in range(2):
                        nc.tensor.matmul(
                            wyp[:], hT1[:, kt, :], wWy[:, kt, :],
                            start=(kt == 0), stop=(kt == 1))
                    if t % 2 == 0:
                        nc.scalar.copy(WyY[:, t, :], wyp[:])
                    else:
                        nc.vector.tensor_copy(WyY[:, t, :], wyp[:])
                    prev_hT1 = hT1

            # ======== Phase 3: attention scan ========
            with (
                tc.tile_pool(name="attn", bufs=1) as ap,
                tc.tile_pool(name="ptree", bufs=1) as ptp,
                tc.tile_pool(name="at_ps", bufs=1, space="PSUM") as aps,
            ):
                e64 = ap.tile([R, LP], f16)
                nc.vector.memset(e64[:], 0.0)
                den = ap.tile([R, 1], f32)
                rden = ap.tile([R, 1], f32)
                al = ap.tile([R, LP], f16)
                s_rl = ap.tile([R, L], f32)
                sm = ap.tile([R, L], f32)

                for t in range(l_attn):
                    # psum [R, 512]: [0:256] accumulates h2@Wha + r@Wra,
                    # [256:512] r@Wta (Wra|Wta concatenated as Wrta)
                    tmpra = aps.tile([R, 2 * H], f32, tag="tmps")
                    for kt in range(2):
                        nc.tensor.matmul(
                            tmpra[:, 0:256], Y2T[:, kt, t, :], wWha[:, kt, :],
                            start=(kt == 0), stop=False)
                    for kt in range(2):
                        nc.tensor.matmul(
                            tmpra[:, 0:256], rT[:, kt, :], wWrta[:, kt, 0:256],
                            start=False, stop=(kt == 1))
                    for kt in range(2):
                        nc.tensor.matmul(
                            tmpra[:, 256:512], rT[:, kt, :], wWrta[:, kt, 256:512],
                            start=(kt == 0), stop=(kt == 1))
                    nc.scalar.activation(TT[:], tmpra[:, 256:512], Act.Tanh)
                    # M = tanh(WyY + tmp); score = reduce_h(M * w)
                    nc.vector.tensor_tensor(
                        MM[:], WyY[:],
                        tmpra[:, 0:256].unsqueeze(1).broadcast_to([R, L, H]),
                        op=Alu.add)
                    mflat = MM[:].rearrange("p l h -> p (l h)")
                    nc.scalar.activation(mflat[:], mflat[:], Act.Tanh)
                    nc.vector.tensor_tensor(
                        MM[:], MM[:],
                        wrow[:].unsqueeze(1).broadcast_to([R, L, H]),
                        op=Alu.mult)
                    nc.vector.tensor_reduce(
                        s_rl[:], MM[:], axis=mybir.AxisListType.X, op=Alu.add)
                    # masked softmax -> alpha
                    nc.vector.tensor_tensor(
                        sm[:], s_rl[:], maskadd[:, 0:L], op=Alu.add)
                    nc.scalar.activation(
                        e64[:, 0:L], sm[:], Act.Exp, accum_out=den[:])
                    nc.vector.reciprocal(rden[:], den[:])
                    nc.vector.tensor_scalar_mul(al[:], e64[:], rden[:])
                    # u = sum_l alpha * Y
                    P = ptp.tile([128, H, LP], f16, tag="P")
                    nc.vector.tensor_tensor(
                        P[:], Yrh[:],
                        al[:].unsqueeze(1).broadcast_to([R, H, LP]),
                        op=Alu.mult)
                    nc.vector.tensor_reduce(
                        uu[:], P[:], axis=mybir.AxisListType.X, op=Alu.add)
                    # r = u + T ; r_L += sel_t * r ; transpose r via xbar
                    nc.vector.tensor_tensor(rr16[:], uu[:], TT[:], op=Alu.add)
                    nc.vector.scalar_tensor_tensor(
                        rL[:], rr16[:], sel[:, t:t + 1], rL[:],
                        op0=Alu.mult, op1=Alu.add)
                    nc.sync.dma_start_transpose(rT[:, 0, :], rr16[:, 0:128])
                    nc.sync.dma_start_transpose(rT[:, 1, :], rr16[:, 128:256])

                # ======== Phase 4: final head ========
                rLT = ap.tile([128, 2, R], f16)
                for kt in range(2):
                    tp = aps.tile([128, 128], f32, tag="rtp")
                    nc.tensor.transpose(
                        tp[:], rL[:, kt * 128:(kt + 1) * 128], ident[:])
                    nc.scalar.copy(rLT[:, kt, :], tp[:])
                fT = ap.tile([128, 2, R], f16)
                for mt in range(2):
                    msl = slice(mt * 128, (mt + 1) * 128)
                    fps = aps.tile([128, R], f32, tag="fps")
                    for kt in range(2):
                        nc.tensor.matmul(
                            fps[:], wWpa[:, kt, msl], rLT[:, kt, :],
                            start=(kt == 0), stop=False)
                    for kt in range(2):
                        nc.tensor.matmul(
                            fps[:], wWxa[:, kt, msl], Y2T[:, kt, L - 1, :],
                            start=False, stop=(kt == 1))
                    nc.scalar.activation(fT[:, mt, :], fps[:], Act.Tanh)
                lhT = ap.tile([128, 2, BC], f16)
                nc.vector.tensor_tensor(
                    lhT[:], fT[:, :, 0:BC], fT[:, :, BC:R], op=Alu.add)
                ops_ = aps.tile([BC, 2], f32, tag="ops")
                for kt in range(2):
                    nc.tensor.matmul(
                        ops_[:], lhT[:, kt, :], wU[:, kt, :],
                        start=(kt == 0), stop=False)
                nc.tensor.matmul(ops_[:], wones[:], wbout[:], start=False, stop=True)
                osb = ap.tile([BC, 2], f32)
                nc.vector.tensor_copy(osb[:], ops_[:])
                nc.sync.dma_start(out_d[:], osb[:])

    return _apply_wait_split(nc)


# gate-column permutation: TF order [i,j,f,o] -> device order [j,i,f,o]
_GPERM = np.concatenate([
    np.arange(256, 512), np.arange(0, 256),
    np.arange(512, 768), np.arange(768, 1024)])


def _prep_inputs(E, Wx1, Wh1, b1, Wx2, Wh2, b2, W_y, Wh_a, Wr_a, w_a, Wt_a,
                 Wp_a, Wxa, U, b_out, input1, input2, seqlen1, seqlen2):
    """Build the per-core input maps (host-side sharding + packing)."""
    f16 = np.float16
    E = np.asarray(E, np.float32)
    qs = NSIG * float(E.std()) / 127.0
    E8 = np.clip(np.round(E * (1.0 / qs)), -127, 127).astype(np.int8)

    def pack_w2(W, perm=None, scale=None):
        W = np.asarray(W, np.float32)
        if perm is not None:
            W = W[:, perm]
        if scale is not None:
            W = W * scale
        return np.stack([W[0:128], W[128:256]], axis=1).astype(f16)

    def packB(W, b):
        W = np.asarray(W, np.float32)[:, _GPERM] * qs
        b = np.asarray(b, np.float32)[_GPERM].copy()
        b[512:768] += 1.0  # TF forget_bias baked into the bias row
        out = np.zeros((DB, H4), np.float32)
        out[0:44] = W[256:300]
        out[44] = b        # bias row is NOT quant-scaled
        return out.astype(f16)

    Wrta = np.concatenate([np.asarray(Wr_a, np.float32),
                           np.asarray(Wt_a, np.float32)], axis=1)
    parts = [
        pack_w2(Wx1, _GPERM, qs).ravel(), pack_w2(Wx2, _GPERM, qs).ravel(),
        packB(Wx1, b1).ravel(), packB(Wx2, b2).ravel(),
        pack_w2(Wh1, _GPERM).ravel(), pack_w2(Wh2, _GPERM).ravel(),
        pack_w2(W_y).ravel(), pack_w2(Wh_a).ravel(), pack_w2(Wrta).ravel(),
        pack_w2(Wp_a).ravel(), pack_w2(Wxa).ravel(),
        pack_w2(U).ravel(),
        np.asarray(b_out, np.float32).reshape(1, 2).astype(f16).ravel(),
        np.asarray(w_a, np.float32).reshape(1, H).astype(f16).ravel(),
    ]
    wflat = np.concatenate(parts)
    assert wflat.size == _off
    wflat = np.concatenate([wflat, np.zeros(SW - wflat.size, f16)])

    input1 = np.asarray(input1)
    input2 = np.asarray(input2)
    seqlen1 = np.asarray(seqlen1)
    seqlen2 = np.asarray(seqlen2)

    in_maps = []
    for c in range(NC):
        sl = slice(c * BC, (c + 1) * BC)
        t1, t2 = input1[sl], input2[sl]
        s1, s2 = seqlen1[sl], seqlen2[sl]
        stack1 = np.concatenate([t1, t2], 0)   # [128, 60] tokens, slot1
        lf = np.concatenate([s1, s2], 0)       # len of first-arg seq per row
        ls = np.concatenate([s2, s1], 0)       # len of second-arg seq per row

        m = {}
        m["x1"] = E8[stack1]                   # [128, 60, 300] row-major int8
        m["sl"] = np.stack([lf, ls - 1], axis=1).astype(np.float32)
        m["wsh"] = wflat[c * SH:(c + 1) * SH]
        in_maps.append(m)
    return in_maps


_last_exec_ns = None


def kernel(__trace=False, **inputs):
    global _last_exec_ns
    from concourse.bass_utils import run_bass_kernel_spmd

    if "nc" not in _cache:
        _cache["nc"] = _build_nc()
    nc = _cache["nc"]
    in_maps = _prep_inputs(**inputs)
    res = run_bass_kernel_spmd(nc, in_maps, core_ids=list(range(NC)),
                               trace=__trace)
    if getattr(res, "exec_time_ns", None):
        _last_exec_ns = res.exec_time_ns
    out = np.concatenate([r["out"] for r in res.results], axis=0)
    return out.astype(np.float32)


# revision 21
# speedup vs baseline: 3.2148x; 1.1103x over previous
"""Trainium2 Bass kernel for the AttentionModel (word-by-word attention entailment model).

Contract: kernel(**inputs) takes FULL unsharded inputs (as produced by
setup_inputs()) and returns the FULL [512, 2] output. Internally the batch is
sharded over 8 NeuronCores (64 sequences each); the two symmetric branches are
stacked on the partition axis so each core processes 128 "rows"
(row r < 64 -> branch1 seq r, row r >= 64 -> branch2 seq r-64).

The end-to-end call on this axon-tunneled setup is dominated by (a) host->
device transfer at ~40-50 MB/s and (b) per-instruction device overhead, so the
design minimizes both payload bytes and instruction count:
  * Only slot-1 embeddings are sent, row-major INT8 [128, L, 300] per core
    (2.3 MB): E is symmetrically quantized at 4.5 sigma and the quant scale is
    folded into the Wx weights host-side; the device just int8->f16 converts.
    Slot 2's stack ([x2;x1]) is the same data with the row axis rotated by 64,
    derived on device. Dims-major tiles for the PE come from DMA-engine (xbar)
    transposes plus one PE transpose for the 44-row tail chunk.
  * All weights live in one flat f16 buffer; each core uploads 1/8 and the
    full buffer is reassembled on device with an AllGather.
  * Gate columns are pre-permuted to [j,i,f,o] with the LSTM forget bias baked
    into the bias row, so the three sigmoids run as ONE activation; gates for
    BOTH LSTM slots accumulate in one f16 PSUM tile [128, 2, 1024] so every
    elementwise/activation op handles both slots in a single instruction.
  * Freeze masks are uint8 + copy_predicated.
  * The attention keeps M row-major [row, l, h]: score = reduce(M*w) is 3 big
    instructions instead of a 65-instruction PSUM-chunk + DRAM-bounce
    pipeline; Wr_a|Wt_a are concatenated so r@Wra and r@Wta share matmuls.

Pipeline per core: two 60-step LSTMs (with inline Y1@W_y), 60-step attention
scan, final head tanh(r_L@Wp_a + h2@Wxa) summed over branches, @U + b_out.
"""

import json

import numpy as np


def _split_multi_waits(raw: bytes) -> bytes:
    """Walrus codegen in this toolchain only encodes one sync-wait per
    instruction. Split every instruction carrying N>1 waits into N-1
    standalone EventSemaphore waits (same engine, program order) followed by
    the original instruction keeping a single wait. Sem conditions are
    monotonic, so a sequential wait chain is equivalent to the combined wait.
    """
    j = json.loads(raw)
    uid = [0]
    for fn in j.get("functions", []):
        for blk in fn.get("blocks", []):
            insts = blk.get("instructions", [])
            out = []
            for inst in insts:
                si = inst.get("sync_info")
                waits = (si or {}).get("on_wait") or []
                if len(waits) > 1:
                    eng = inst.get("engine")
                    for w in waits[:-1]:
                        uid[0] += 1
                        out.append({
                            "debug": inst.get("debug", 0),
                            "engine": eng,
                            "ins": [],
                            "outs": [],
                            "name": f"WSPLIT-{uid[0]}",
                            "opcode": "EventSemaphore",
                            "sync_info": {"on_update": [], "on_wait": [w]},
                        })
                    si["on_wait"] = [waits[-1]]
                out.append(inst)
            blk["instructions"] = out
    return json.dumps(j).encode()


def _apply_wait_split(nc):
    import concourse.bass as bass

    patched = _split_multi_waits(bass.Bass.to_json_bytes(nc))
    nc.to_json_bytes = lambda: patched
    return nc


B, L, D, H, V = 512, 60, 300, 256, 50000
NC = 8                 # cores
BC = B // NC           # 64 sequences per core
R = 2 * BC             # 128 rows (2 branches)
H4 = 4 * H             # 1024
DB = 45                # third d-chunk: rows 256..299 + bias ones-row at 44
LP = 64                # l padded to 64 for the alpha broadcast
NEG = -10000.0
NSIG = 4.5             # int8 clip point for the embedding quantization

# flat weight buffer layout (f16 elems); uploaded sharded + AllGathered
_WSPECS = [
    ("Wx1A", (128, 2, H4)), ("Wx2A", (128, 2, H4)),
    ("Wx1B", (DB, H4)), ("Wx2B", (DB, H4)),
    ("Wh1", (128, 2, H4)), ("Wh2", (128, 2, H4)),
    ("Wy", (128, 2, H)), ("Wha", (128, 2, H)), ("Wrta", (128, 2, 2 * H)),
    ("Wpa", (128, 2, H)), ("Wxa", (128, 2, H)),
    ("U", (128, 2, 2)), ("bout", (1, 2)), ("wrow", (1, H)),
]
_WOFF = {}
_off = 0
for _nm, _shp in _WSPECS:
    _WOFF[_nm] = _off
    _n = 1
    for _d in _shp:
        _n *= _d
    _off += _n
SH = 192000            # per-core weight shard elems
SW = SH * NC           # padded flat weight buffer elems
assert _off <= SW

_cache = {}


def _build_nc(l_lstm=L, l_attn=L):
    import concourse.bass as bass
    import concourse.mybir as mybir
    import concourse.tile as tile
    from concourse.masks import make_identity

    f32 = mybir.dt.float32
    f16 = mybir.dt.float16
    u8 = mybir.dt.uint8
    i8 = mybir.dt.int8
    Alu = mybir.AluOpType
    Act = mybir.ActivationFunctionType

    nc = bass.Bass()

    # ---------------- DRAM I/O ----------------
    x1_d = nc.dram_tensor("x1", [R, L, D], i8, kind="ExternalInput")
    wsh_d = nc.dram_tensor("wsh", [SH], f16, kind="ExternalInput")
    sl_d = nc.dram_tensor("sl", [R, 2], f32, kind="ExternalInput")
    out_d = nc.dram_tensor("out", [BC, 2], f32, kind="ExternalOutput")

    with tile.TileContext(nc) as tc:
        with (
            tc.tile_pool(name="persist", bufs=1) as pp,
            tc.tile_pool(name="dram", bufs=1, space="DRAM") as dp,
        ):
            # ---- weight shard upload + AllGather into the full flat buffer
            wshard = dp.tile([SH], f16)
            wfull = dp.tile([SW], f16)
            nc.gpsimd.dma_start(wshard[:], wsh_d[:])
            nc.gpsimd.collective_compute(
                "AllGather", mybir.AluOpType.bypass,
                replica_groups=[list(range(NC))],
                ins=[wshard[:].opt()], outs=[wfull[:].opt()])

            def wslice(name):
                off = _WOFF[name]
                shp = dict(_WSPECS)[name]
                n = 1
                for d_ in shp:
                    n *= d_
                ap = wfull[off:off + n]
                if len(shp) == 2:
                    return ap.rearrange("(p n) -> p n", p=shp[0])
                return ap.rearrange("(p k n) -> p k n", p=shp[0], k=shp[1])

            # persistent sbuf tiles
            Y2T = pp.tile([128, 2, L, R], f16)    # slot2 h-state transposed, per t
            Yrh = pp.tile([128, H, LP], f16)      # slot1 h row-major [row, h, l]
            WyY = pp.tile([128, L, H], f16)       # Y1 @ W_y row-major [row, l, h]
            MM = pp.tile([128, L, H], f16)        # attention M buffer
            wWy = pp.tile([128, 2, H], f16)
            wWha = pp.tile([128, 2, H], f16)
            wWrta = pp.tile([128, 2, 2 * H], f16)
            wWpa = pp.tile([128, 2, H], f16)
            wWxa = pp.tile([128, 2, H], f16)
            wU = pp.tile([128, 2, 2], f16)
            wbout = pp.tile([1, 2], f16)
            wones = pp.tile([1, BC], f16)
            wones1 = pp.tile([1, 128], f16)
            wrow = pp.tile([128, H], f16)         # w_a replicated on partitions
            sl_sb = pp.tile([R, 2], f32)
            lio = pp.tile([R, LP], f32)
            maskadd = pp.tile([R, LP], f16)
            sel = pp.tile([R, LP], f32)
            mfu = pp.tile([R, 2, LP], u8)         # freeze masks, both slots
            ident = pp.tile([128, 128], f32)
            ident16 = pp.tile([128, 128], f16)
            # states
            rr16 = pp.tile([R, H], f16)           # r (row major)
            rT = pp.tile([128, 2, R], f16)        # r transposed
            rL = pp.tile([R, H], f32)
            uu = pp.tile([R, H], f32)
            TT = pp.tile([R, H], f32)

            make_identity(nc, ident[:])
            make_identity(nc, ident16[:])
            for t_ in (Yrh, rT):
                nc.vector.memset(t_[:], 0.0)
            nc.vector.memset(rL[:], 0.0)
            nc.vector.memset(wones[:], 1.0)
            nc.vector.memset(wones1[:], 1.0)

            for dst, nm in [
                (wWy, "Wy"), (wWha, "Wha"), (wWrta, "Wrta"),
                (wWpa, "Wpa"), (wWxa, "Wxa"), (wU, "U"), (wbout, "bout"),
            ]:
                nc.sync.dma_start(dst[:], wslice(nm))

            # ---- w_a replicated across partitions via ones-matmul
            with tc.tile_pool(name="init_ps", bufs=1, space="PSUM") as ips:
                wr_sb = pp.tile([1, H], f16)
                nc.sync.dma_start(wr_sb[:], wslice("wrow"))
                wp = ips.tile([128, H], f32, tag="wp")
                nc.tensor.matmul(wp[:], wones1[:], wr_sb[:], start=True, stop=True)
                nc.scalar.copy(wrow[:], wp[:])

            # ---- masks from seqlens: lf = sl[:,0], ls-1 = sl[:,1]
            nc.sync.dma_start(sl_sb[:], sl_d[:])
            nc.gpsimd.iota(lio[:], pattern=[[1, LP]], base=0,
                           channel_multiplier=0,
                           allow_small_or_imprecise_dtypes=True)
            nc.vector.tensor_scalar(
                mfu[:, 0, :], lio[:], sl_sb[:, 0:1], None, op0=Alu.is_lt)
            nc.vector.tensor_scalar(
                mfu[:, 1, :], lio[:], sl_sb[:, 1:2], None, op0=Alu.is_le)
            nc.vector.tensor_scalar(
                maskadd[:], lio[:], sl_sb[:, 0:1], NEG,
                op0=Alu.is_ge, op1=Alu.mult)
            nc.vector.tensor_scalar(
                sel[:], lio[:], sl_sb[:, 1:2], None, op0=Alu.is_equal)

            # ======== Phase 1: the two LSTMs (+ inline Y1 @ W_y) ========
            with (
                tc.tile_pool(name="lstm", bufs=1) as lp,
                tc.tile_pool(name="lstm_xq", bufs=3) as lxq,
                tc.tile_pool(name="lstm_xt", bufs=2) as lxt,
                tc.tile_pool(name="lstm_ps", bufs=1, space="PSUM") as lps,
                tc.tile_pool(name="xtr_ps", bufs=2, space="PSUM") as xps,
                tc.tile_pool(name="wyy_ps", bufs=1, space="PSUM") as wps,
            ):
                wWx1A = lp.tile([128, 2, H4], f16, name="wx1a")
                wWx2A = lp.tile([128, 2, H4], f16, name="wx2a")
                wWx1B = lp.tile([DB, H4], f16, name="wx1b")
                wWx2B = lp.tile([DB, H4], f16, name="wx2b")
                wWh1 = lp.tile([128, 2, H4], f16, name="wh1")
                wWh2 = lp.tile([128, 2, H4], f16, name="wh2")
                for dst, nm in [(wWx1A, "Wx1A"), (wWx2A, "Wx2A"),
                                (wWx1B, "Wx1B"), (wWx2B, "Wx2B"),
                                (wWh1, "Wh1"), (wWh2, "Wh2")]:
                    nc.sync.dma_start(dst[:], wslice(nm))

                wWxA = {1: wWx1A, 2: wWx2A}
                wWxB = {1: wWx1B, 2: wWx2B}
                wWh = {1: wWh1, 2: wWh2}

                cc2 = lp.tile([R, 2, H], f32, name="cc2")   # cell state, both slots
                hh2 = lp.tile([R, 2, H], f16, name="hh2")   # hidden, both slots
                nc.vector.memset(cc2[:], 0.0)
                nc.vector.memset(hh2[:], 0.0)

                # pre-set both xt1 pool buffers to 1.0: per-step writes cover
                # chunks 0/1 fully and chunk-2 rows 0:44, so the bias ones-row
                # (row 44 of chunk 2) persists; the rotated copy for slot 2
                # carries it over
                for _ in range(2):
                    b_ = lxt.tile([128, 3, R], f16, tag="xt1")
                    nc.vector.memset(b_[:], 1.0)

                prev_hT1 = None
                xq16 = None
                for t in range(l_lstm):
                    # int8 x loads + f16 convert batched over 4 steps
                    if t % 4 == 0:
                        nt = min(4, l_lstm - t)
                        xq8 = lxq.tile([R, 4, D], i8, tag="xq8")
                        nc.gpsimd.dma_start(xq8[:, 0:nt, :], x1_d[:, t:t + nt, :])
                        xq16 = lxq.tile([R, 4, D], f16, tag="xq16")
                        nc.gpsimd.tensor_copy(xq16[:, 0:nt, :], xq8[:, 0:nt, :])
                    # slot-1 x_t dims-major [128, 3, R]: xbar-transpose the two
                    # 128-row d-chunks, PE-transpose the 44-row tail
                    xall1 = lxt.tile([128, 3, R], f16, tag="xt1")
                    nc.sync.dma_start_transpose(
                        xall1[:, 0, :], xq16[:, t % 4, 0:128])
                    nc.sync.dma_start_transpose(
                        xall1[:, 1, :], xq16[:, t % 4, 128:256])
                    tpx = xps.tile([128, 128], f16, tag="xtp")
                    nc.tensor.transpose(
                        tpx[0:44, :], xq16[:, t % 4, 256:300], ident16[:])
                    nc.scalar.copy(xall1[0:44, 2, :], tpx[0:44, :])
                    # slot-2 x_t = slot-1 rotated by 64 on the row axis
                    xall2 = lxt.tile([128, 3, R], f16, tag="xt2")
                    nc.vector.tensor_copy(xall2[:, :, 0:BC], xall1[:, :, BC:R])
                    nc.gpsimd.tensor_copy(xall2[:, :, BC:R], xall1[:, :, 0:BC])
                    xts = {1: xall1, 2: xall2}
                    hT1 = lxt.tile([128, 2, R], f16, tag="hT1")
                    # gates for BOTH slots in one f32 psum tile [R, 2, 1024]
                    gps = lps.tile([R, 2, H4], f32, tag="gates")
                    for s in (1, 2):
                        for nck in range(2):
                            nsl = slice(nck * 512, (nck + 1) * 512)
                            mms = [(xts[s][:, 0, :], wWxA[s][:, 0, nsl]),
                                   (xts[s][:, 1, :], wWxA[s][:, 1, nsl]),
                                   (xts[s][0:DB, 2, :], wWxB[s][:, nsl])]
                            if t > 0:
                                hTs = [prev_hT1[:, kt_, :] for kt_ in range(2)] \
                                    if s == 1 else \
                                    [Y2T[:, kt_, t - 1, :] for kt_ in range(2)]
                                mms += [(hT, wWh[s][:, kt_, nsl])
                                        for kt_, hT in enumerate(hTs)]
                            for i, (a_, b_) in enumerate(mms):
                                nc.tensor.matmul(
                                    gps[:, s - 1, nsl], a_, b_,
                                    start=(i == 0), stop=(i == len(mms) - 1))
                    # gates pre-permuted to [j, i, f, o]; f bias baked.
                    # process BOTH slots per instruction via [R, 2, *] APs
                    tj = lp.tile([R, 2, H], f32, tag="tj")
                    sio = lp.tile([R, 2, 3 * H], f32, tag="sio")
                    nc.scalar.activation(tj[:], gps[:, :, 0:256], Act.Tanh)
                    nc.scalar.activation(sio[:], gps[:, :, 256:1024], Act.Sigmoid)
                    t1 = lp.tile([R, 2, H], f32, tag="t1")
                    t2 = lp.tile([R, 2, H], f32, tag="t2")
                    cn = lp.tile([R, 2, H], f32, tag="cn")
                    nc.vector.tensor_tensor(
                        t1[:], cc2[:], sio[:, :, 256:512], op=Alu.mult)
                    nc.gpsimd.tensor_tensor(
                        t2[:], tj[:], sio[:, :, 0:256], op=Alu.mult)
                    nc.vector.tensor_tensor(cn[:], t1[:], t2[:], op=Alu.add)
                    nc.vector.copy_predicated(
                        cc2[:], mfu[:, :, t:t + 1].broadcast_to([R, 2, H]), cn[:])
                    tcn = lp.tile([R, 2, H], f32, tag="tcn")
                    nc.scalar.activation(tcn[:], cn[:], Act.Tanh)
                    hn = lp.tile([R, 2, H], f16, tag="hn")
                    nc.gpsimd.tensor_tensor(
                        hn[:], tcn[:], sio[:, :, 512:768], op=Alu.mult)
                    nc.vector.copy_predicated(
                        hh2[:], mfu[:, :, t:t + 1].broadcast_to([R, 2, H]), hn[:])
                    # transpose frozen h via xbar DMA
                    nc.sync.dma_start_transpose(hT1[:, 0, :], hh2[:, 0, 0:128])
                    nc.sync.dma_start_transpose(hT1[:, 1, :], hh2[:, 0, 128:256])
                    nc.sync.dma_start_transpose(Y2T[:, 0, t, :], hh2[:, 1, 0:128])
                    nc.sync.dma_start_transpose(Y2T[:, 1, t, :], hh2[:, 1, 128:256])
                    nc.gpsimd.tensor_copy(Yrh[:, :, t], hh2[:, 0, :])
                    # inline WyY[:, t, :] = Y1_t @ W_y
                    wyp = wps.tile([R, H], f32, tag="wyy")
                    for kt in range(2):
                        nc.tensor.matmul(
                            wyp[:], hT1[:, kt, :], wWy[:, kt, :],
                            start=(kt == 0), stop=(kt == 1))
                    if t % 2 == 0:
                        nc.scalar.copy(WyY[:, t, :], wyp[:])
                    else:
                        nc.vector.tensor_copy(WyY[:, t, :], wyp[:])
                    prev_hT1 = hT1

            # ======== Phase 3: attention scan ========
            with (
                tc.tile_pool(name="attn", bufs=1) as ap,
                tc.tile_pool(name="ptree", bufs=1) as ptp,
                tc.tile_pool(name="at_ps", bufs=1, space="PSUM") as aps,
            ):
                e64 = ap.tile([R, LP], f16)
                nc.vector.memset(e64[:], 0.0)
                den = ap.tile([R, 1], f32)
                rden = ap.tile([R, 1], f32)
                s_rl = ap.tile([R, L], f32)
                sm = ap.tile([R, L], f32)

                for t in range(l_attn):
                    # psum [R, 512]: [0:256] accumulates h2@Wha + r@Wra,
                    # [256:512] r@Wta (Wra|Wta concatenated as Wrta)
                    tmpra = aps.tile([R, 2 * H], f32, tag="tmps")
                    for kt in range(2):
                        nc.tensor.matmul(
                            tmpra[:, 0:256], Y2T[:, kt, t, :], wWha[:, kt, :],
                            start=(kt == 0), stop=False)
                    for kt in range(2):
                        nc.tensor.matmul(
                            tmpra[:, 0:256], rT[:, kt, :], wWrta[:, kt, 0:256],
                            start=False, stop=(kt == 1))
                    for kt in range(2):
                        nc.tensor.matmul(
                            tmpra[:, 256:512], rT[:, kt, :], wWrta[:, kt, 256:512],
                            start=(kt == 0), stop=(kt == 1))
                    nc.scalar.activation(TT[:], tmpra[:, 256:512], Act.Tanh)
                    # M = tanh(WyY + tmp); score = reduce_h(M * w)
                    nc.vector.tensor_tensor(
                        MM[:], WyY[:],
                        tmpra[:, 0:256].unsqueeze(1).broadcast_to([R, L, H]),
                        op=Alu.add)
                    mflat = MM[:].rearrange("p l h -> p (l h)")
                    nc.scalar.activation(mflat[:], mflat[:], Act.Tanh)
                    nc.vector.tensor_tensor(
                        MM[:], MM[:],
                        wrow[:].unsqueeze(1).broadcast_to([R, L, H]),
                        op=Alu.mult)
                    nc.vector.tensor_reduce(
                        s_rl[:], MM[:], axis=mybir.AxisListType.X, op=Alu.add)
                    # masked softmax -> alpha
                    nc.vector.tensor_tensor(
                        sm[:], s_rl[:], maskadd[:, 0:L], op=Alu.add)
                    nc.scalar.activation(
                        e64[:, 0:L], sm[:], Act.Exp, accum_out=den[:])
                    nc.vector.reciprocal(rden[:], den[:])
                    # u_unnorm = sum_l exp * Y ; normalization folded into r
                    P = ptp.tile([128, H, LP], f16, tag="P")
                    nc.vector.tensor_tensor(
                        P[:], Yrh[:],
                        e64[:].unsqueeze(1).broadcast_to([R, H, LP]),
                        op=Alu.mult)
                    nc.vector.tensor_reduce(
                        uu[:], P[:], axis=mybir.AxisListType.X, op=Alu.add)
                    # r = u*rden + T ; r_L += sel_t * r ; transpose r via xbar
                    nc.vector.scalar_tensor_tensor(
                        rr16[:], uu[:], rden[:], TT[:],
                        op0=Alu.mult, op1=Alu.add)
                    nc.vector.scalar_tensor_tensor(
                        rL[:], rr16[:], sel[:, t:t + 1], rL[:],
                        op0=Alu.mult, op1=Alu.add)
                    nc.sync.dma_start_transpose(rT[:, 0, :], rr16[:, 0:128])
                    nc.sync.dma_start_transpose(rT[:, 1, :], rr16[:, 128:256])

                # ======== Phase 4: final head ========
                rLT = ap.tile([128, 2, R], f16)
                for kt in range(2):
                    tp = aps.tile([128, 128], f32, tag="rtp")
                    nc.tensor.transpose(
                        tp[:], rL[:, kt * 128:(kt + 1) * 128], ident[:])
                    nc.scalar.copy(rLT[:, kt, :], tp[:])
                fT = ap.tile([128, 2, R], f16)
                for mt in range(2):
                    msl = slice(mt * 128, (mt + 1) * 128)
                    fps = aps.tile([128, R], f32, tag="fps")
                    for kt in range(2):
                        nc.tensor.matmul(
                            fps[:], wWpa[:, kt, msl], rLT[:, kt, :],
                            start=(kt == 0), stop=False)
                    for kt in range(2):
                        nc.tensor.matmul(
                            fps[:], wWxa[:, kt, msl], Y2T[:, kt, L - 1, :],
                            start=False, stop=(kt == 1))
                    nc.scalar.activation(fT[:, mt, :], fps[:], Act.Tanh)
                lhT = ap.tile([128, 2, BC], f16)
                nc.vector.tensor_tensor(
                    lhT[:], fT[:, :, 0:BC], fT[:, :, BC:R], op=Alu.add)
                ops_ = aps.tile([BC, 2], f32, tag="ops")
                for kt in range(2):
                    nc.tensor.matmul(
                        ops_[:], lhT[:, kt, :], wU[:, kt, :],
                        start=(kt == 0), stop=False)
                nc.tensor.matmul(ops_[:], wones[:], wbout[:], start=False, stop=True)
                osb = ap.tile([BC, 2], f32)
                nc.vector.tensor_copy(osb[:], ops_[:])
                nc.sync.dma_start(out_d[:], osb[:])

    return _apply_wait_split(nc)


# gate-column permutation: TF order [i,j,f,o] -> device order [j,i,f,o]
_GPERM = np.concatenate([
    np.arange(256, 512), np.arange(0, 256),
    np.arange(512, 768), np.arange(768, 1024)])


def _prep_inputs(E, Wx1, Wh1, b1, Wx2, Wh2, b2, W_y, Wh_a, Wr_a, w_a, Wt_a,
                 Wp_a, Wxa, U, b_out, input1, input2, seqlen1, seqlen2):
    """Build the per-core input maps (host-side sharding + packing)."""
    f16 = np.float16
    E = np.asarray(E, np.float32)
    qs = NSIG * float(E.std()) / 127.0
    E8 = np.clip(np.round(E * (1.0 / qs)), -127, 127).astype(np.int8)

    def pack_w2(W, perm=None, scale=None):
        W = np.asarray(W, np.float32)
        if perm is not None:
            W = W[:, perm]
        if scale is not None:
            W = W * scale
        return np.stack([W[0:128], W[128:256]], axis=1).astype(f16)

    def packB(W, b):
        W = np.asarray(W, np.float32)[:, _GPERM] * qs
        b = np.asarray(b, np.float32)[_GPERM].copy()
        b[512:768] += 1.0  # TF forget_bias baked into the bias row
        out = np.zeros((DB, H4), np.float32)
        out[0:44] = W[256:300]
        out[44] = b        # bias row is NOT quant-scaled
        return out.astype(f16)

    Wrta = np.concatenate([np.asarray(Wr_a, np.float32),
                           np.asarray(Wt_a, np.float32)], axis=1)
    parts = [
        pack_w2(Wx1, _GPERM, qs).ravel(), pack_w2(Wx2, _GPERM, qs).ravel(),
        packB(Wx1, b1).ravel(), packB(Wx2, b2).ravel(),
        pack_w2(Wh1, _GPERM).ravel(), pack_w2(Wh2, _GPERM).ravel(),
        pack_w2(W_y).ravel(), pack_w2(Wh_a).ravel(), pack_w2(Wrta).ravel(),
        pack_w2(Wp_a).ravel(), pack_w2(Wxa).ravel(),
        pack_w2(U).ravel(),
        np.asarray(b_out, np.float32).reshape(1, 2).astype(f16).ravel(),
        np.asarray(w_a, np.float32).reshape(1, H).astype(f16).ravel(),
    ]
    wflat = np.concatenate(parts)
    assert wflat.size == _off
    wflat = np.concatenate([wflat, np.zeros(SW - wflat.size, f16)])

    input1 = np.asarray(input1)
    input2 = np.asarray(input2)
    seqlen1 = np.asarray(seqlen1)
    seqlen2 = np.asarray(seqlen2)

    in_maps = []
    for c in range(NC):
        sl = slice(c * BC, (c + 1) * BC)
        t1, t2 = input1[sl], input2[sl]
        s1, s2 = seqlen1[sl], seqlen2[sl]
        stack1 = np.concatenate([t1, t2], 0)   # [128, 60] tokens, slot1
        lf = np.concatenate([s1, s2], 0)       # len of first-arg seq per row
        ls = np.concatenate([s2, s1], 0)       # len of second-arg seq per row

        m = {}
        m["x1"] = E8[stack1]                   # [128, 60, 300] row-major int8
        m["sl"] = np.stack([lf, ls - 1], axis=1).astype(np.float32)
        m["wsh"] = wflat[c * SH:(c + 1) * SH]
        in_maps.append(m)
    return in_maps


_last_exec_ns = None


def kernel(__trace=False, **inputs):
    global _last_exec_ns
    from concourse.bass_utils import run_bass_kernel_spmd

    if "nc" not in _cache:
        _cache["nc"] = _build_nc()
    nc = _cache["nc"]
    in_maps = _prep_inputs(**inputs)
    res = run_bass_kernel_spmd(nc, in_maps, core_ids=list(range(NC)),
                               trace=__trace)
    if getattr(res, "exec_time_ns", None):
        _last_exec_ns = res.exec_time_ns
    out = np.concatenate([r["out"] for r in res.results], axis=0)
    return out.astype(np.float32)
